# revision 69
# baseline (speedup 1.0000x reference)
"""Trainium2 Bass kernel for one dense transformer block.

Full (unsharded) IO: x [4, 2048, 1024] -> out [4, 2048, 1024].
Sharding: 8 cores = 4 batches x 2 query sets. Each core owns one batch's K/V
(2048 rows) and 1024 query rows. Set 0 takes even 256-row blocks {0,2,4,6},
set 1 odd blocks {1,3,5,7}. Set-1 cores store their keys with the two
256-halves of every 512-token chunk swapped, so every core's own query rows
sit at columns [0:256) of each 512-chunk of its key layout -- the Q
projection reads the LN output directly (no separate q-side LayerNorm), and
the causal boundary tiles sit at the same slot positions on every core
(masks are per-core data; the instruction stream is identical).

Numerics: fp8e4 DoubleRow matmuls (0.5 cyc/col, 256-deep contraction) for
Q/K/V/AV/proj/FFN; S stays bf16. Causal masking is additive: a -30 bias is
preloaded into the S psum via one fp8-DR identity matmul, so exp() output
is written as fp8 directly and feeds DoubleRow AV matmuls. The softmax
denominator rides as a ones-column in the V tiles (padded to 128-wide DR
weights; the pad rows' outputs are never read); normalization is a DVE
reciprocal + PE broadcast + DVE multiply per (head, query-block). proj/FFN2
biases are added via rank-1 bf16 matmuls into the psum. The post-attention
tail (proj -> LN2 -> FFN) is column-pipelined so LN2 chains overlap proj/FFN
matmuls.
"""

import sys

sys.path.insert(0, "/opt/trn_rl_repo")

import numpy as np

import concourse.bass as bass
import concourse.mybir as mybir
import concourse.tile as tile
from concourse.bass_utils import run_bass_kernel_spmd

f32 = mybir.dt.float32
f32r = mybir.dt.float32r
bf16 = mybir.dt.bfloat16
fp8 = mybir.dt.float8e4
AL = mybir.AluOpType
AF = mybir.ActivationFunctionType
DR = mybir.MatmulPerfMode.DoubleRow

B, T, C = 4, 2048, 1024
H, D = 16, 64
F = 4 * C
P = 128
TQ = 1024            # query rows per core
LN_EPS = 1e-5
H8 = 8.0             # LN-output fp8 scale
WK8, WQ8, WV8 = 64.0, 512.0, 64.0
WP8, W18, W28 = 64.0, 16.0, 32.0


def _split_sync_waits(nc):
    """This container's walrus supports one sync-wait per instruction; Tile
    emits up to ~3. Hoist extras onto NoOps inserted before the owner."""
    ctr = 0
    for fn in nc.m.functions:
        for bb in fn.blocks:
            out, changed = [], False
            for ins in bb.instructions:
                si = ins.sync_info
                waits = list(si.on_wait) if si is not None and si.on_wait else []
                if len(waits) > 1:
                    changed = True
                    for w in waits[:-1]:
                        ctr += 1
                        nop = mybir.InstNoOp(name=f"waitsplit_{ctr}", ins=[], outs=[])
                        nop.engine = ins.engine
                        nop.sync_info = mybir.SyncInfo(on_wait=[w], on_update=[])
                        out.append(nop)
                        nc.register_instruction(nop, overwrite=True)
                    ins.sync_info = mybir.SyncInfo(
                        on_wait=[waits[-1]], on_update=list(si.on_update or [])
                    )
                out.append(ins)
            if changed:
                bb.instructions = out


def build_program():
    nc = bass.Bass()
    xkvt_d = nc.dram_tensor("xkvt", [8, P, T], bf16, kind="ExternalInput")
    xqres_d = nc.dram_tensor("xqres", [8, P, TQ], bf16, kind="ExternalInput")
    sbias_d = nc.dram_tensor("sbias", [P, 2, 1024], fp8, kind="ExternalInput")
    ipair_d = nc.dram_tensor("ipair", [P, 2, P], fp8, kind="ExternalInput")
    ones128_d = nc.dram_tensor("ones128", [1, P], f32r, kind="ExternalInput")
    eights128_d = nc.dram_tensor("eights128", [1, P], f32r, kind="ExternalInput")
    onescol_bf_d = nc.dram_tensor("onescolbf", [P, 1], bf16, kind="ExternalInput")
    sel2_d = nc.dram_tensor("sel2", [2, P], f32r, kind="ExternalInput")
    onesrow_d = nc.dram_tensor("onesrow", [1, 512], bf16, kind="ExternalInput")
    bprow_d = nc.dram_tensor("bprow", [1, 8, P], bf16, kind="ExternalInput")
    b2row_d = nc.dram_tensor("b2row", [1, 8, P], bf16, kind="ExternalInput")
    wq_d = nc.dram_tensor("wq", [4, P, 8, 256], fp8, kind="ExternalInput")
    wk_d = nc.dram_tensor("wk", [4, P, 8, 256], fp8, kind="ExternalInput")
    wv_d = nc.dram_tensor("wv", [4, P, 8, 256], fp8, kind="ExternalInput")
    wp_d = nc.dram_tensor("wp", [P, 8, C], fp8, kind="ExternalInput")
    w1_d = nc.dram_tensor("w1", [32, P, 8, P], fp8, kind="ExternalInput")
    w2_d = nc.dram_tensor("w2", [8, P, 32, P], fp8, kind="ExternalInput")
    ball_d = nc.dram_tensor("ball", [P, 48], f32, kind="ExternalInput")
    y_d = nc.dram_tensor("y", [C, TQ], f32, kind="ExternalOutput")

    with tile.TileContext(nc) as tc:
        with tc.tile_pool(name="consts", bufs=1) as cpool, \
             tc.tile_pool(name="pers", bufs=1) as pers, \
             tc.tile_pool(name="proj_in", bufs=1) as prpool:
            ones128 = cpool.tile([1, P], f32r)
            nc.sync.dma_start(ones128, ones128_d[:, :])
            eights128 = cpool.tile([1, P], f32r)
            nc.sync.dma_start(eights128, eights128_d[:, :])
            onescol_bf = cpool.tile([P, 1], bf16)
            nc.sync.dma_start(onescol_bf, onescol_bf_d[:, :])
            sel2 = cpool.tile([2, P], f32r)
            nc.sync.dma_start(sel2, sel2_d[:, :])
            onesrow = cpool.tile([1, 512], bf16)
            nc.sync.dma_start(onesrow, onesrow_d[:, :])
            bprow = cpool.tile([1, 8, P], bf16)
            nc.sync.dma_start(bprow, bprow_d[:, :, :])
            b2row = cpool.tile([1, 8, P], bf16)
            nc.sync.dma_start(b2row, b2row_d[:, :, :])
            ipair = cpool.tile([P, 2, P], fp8)
            nc.sync.dma_start(ipair, ipair_d[:, :, :])
            sbias = cpool.tile([P, 2, 1024], fp8)
            eps1 = cpool.tile([1, 1], f32)
            nc.vector.memset(eps1, LN_EPS)
            ball = cpool.tile([P, 48], f32)
            bq_sb = ball[:, 0:8]
            bk_sb = ball[:, 8:16]
            b1_sb = ball[:, 16:48]

            OT8 = pers.tile([P, 8, TQ], fp8)       # attn out / den, fp8

            def ln_stats(src, cols, lnps, lnsm, lnsb, ptag=None, big_pool=None,
                         sq_dve=False, sqp=None):
                """Sum / sum-of-squares ones-matmuls over one 512-col chunk
                of a [P, 8, N] bf16 tensor, then the small chain down to the
                rstd / mean*rstd row vectors."""
                if big_pool is not None:
                    psum_smt = big_pool.tile([P, 4, 256], f32, tag="s",
                                             name="psum_smt")
                    psum_sqt = big_pool.tile([P, 4, 256], f32, tag="s",
                                             name="psum_sqt")
                    psum_sm = psum_smt[0:1, 0:2, :]
                    psum_sq = psum_sqt[0:1, 0:2, :]
                else:
                    psum_sm = lnps.tile([1, 512], f32, tag=ptag or "sm",
                                        name="psum_sm")
                    psum_sq = lnps.tile([1, 512], f32, tag=ptag or "sq",
                                        name="psum_sq")
                for ft in range(8):
                    sq = (sqp or lnsb).tile([P, 512], bf16, tag="sq")
                    if sq_dve:
                        nc.vector.tensor_tensor(
                            sq, src[:, ft, cols], src[:, ft, cols], AL.mult)
                    else:
                        nc.scalar.activation(sq, src[:, ft, cols], AF.Square)
                    nc.tensor.matmul(psum_sm, onescol_bf, src[:, ft, cols],
                                     start=(ft == 0), stop=(ft == 7))
                    nc.tensor.matmul(psum_sq, onescol_bf, sq,
                                     start=(ft == 0), stop=(ft == 7))
                mean = lnsm.tile([1, 512], f32, tag="mean")
                nc.vector.tensor_scalar_mul(mean, psum_sm, 1.0 / C)
                msq = lnsm.tile([1, 512], f32, tag="msq")
                nc.vector.tensor_scalar_mul(msq, psum_sq, 1.0 / C)
                var = lnsm.tile([1, 512], f32, tag="var")
                nc.vector.tensor_tensor(var, mean, mean, AL.mult)
                nc.vector.tensor_tensor(var, msq, var, AL.subtract)
                rrow = lnsm.tile([1, 512], f32r, tag="rrow")
                nc.scalar.activation(rrow, var, AF.Sqrt, bias=eps1, scale=1.0)
                with nc.allow_low_precision(reason="f32r has f32 bits"):
                    nc.vector.reciprocal(rrow, rrow)
                mrow = lnsm.tile([1, 512], f32r, tag="mrow")
                nc.vector.tensor_tensor(mrow, mean, rrow, AL.mult)
                return rrow, mrow

            def ln_finish(src, dst, cols, rows, lnps, lnsb, ptag=None,
                          pool_split=True):
                """Broadcast the (x8) row vectors along partitions and apply;
                dst is fp8 (x8-scaled LN output)."""
                rrow, mrow = rows
                psum_r = lnps.tile([P, 512], f32, tag=ptag or "bc",
                                   name="psum_r")
                nc.tensor.matmul(psum_r, eights128, rrow, start=True, stop=True)
                psum_m = lnps.tile([P, 512], f32, tag=ptag or "bc",
                                   name="psum_m")
                nc.tensor.matmul(psum_m, eights128, mrow, start=True, stop=True)
                rbc = lnsb.tile([P, 512], bf16, tag="rbc")
                nc.scalar.copy(rbc, psum_r)
                mbc = lnsb.tile([P, 512], bf16, tag="mbc")
                nc.scalar.copy(mbc, psum_m)
                for ft in range(8):
                    tmp = lnsb.tile([P, 512], bf16, tag="tmp")
                    k = 2 if pool_split else 4
                    eng = nc.gpsimd if ft % k == 0 else nc.vector
                    eng.tensor_tensor(
                        tmp, src[:, ft, cols], rbc, AL.mult)
                    eng2 = nc.gpsimd if ft % k == k - 1 else nc.vector
                    eng2.tensor_tensor(
                        dst[:, ft, cols], tmp, mbc, AL.subtract)

            # ------------- Phase 0: load xT; LN1 -> h8 (fp8, x8) -------------
            with tc.tile_pool(name="xt_sb", bufs=1) as xtpool:
                wp_t = prpool.tile([P, 8, C], fp8)
                xqT = prpool.tile([P, 8, TQ], bf16)
                xTc = [xtpool.tile([P, 8, 512], bf16, name=f"xtc{cc}")
                       for cc in range(4)]
                h8c = [prpool.tile([P, 8, 512], fp8, name=f"h8c{cc}")
                       for cc in range(4)]

                # ---------------- Phase A: attention ----------------
                with tc.tile_pool(name="wk_p", bufs=2) as wkp, \
                     tc.tile_pool(name="wq_p", bufs=2) as wqp, \
                     tc.tile_pool(name="wv_p", bufs=1) as wvp, \
                     tc.tile_pool(name="kq_big", bufs=2) as gpool, \
                     tc.tile_pool(name="v_sb", bufs=2) as vpool, \
                     tc.tile_pool(name="pt_sb", bufs=12) as ptpool, \
                     tc.tile_pool(name="den_sb", bufs=1) as dpool, \
                     tc.tile_pool(name="ps_ab", bufs=2, space="PSUM") as ps_ab, \
                     tc.tile_pool(name="ps_s", bufs=2, space="PSUM") as ps_s, \
                     tc.tile_pool(name="ps_o", bufs=2, space="PSUM") as ps_o:

                    def make_units(gp):
                        """Prefetch gp's weights now; return (tiles, unit
                        closures) emitting gp's K/Q/V DoubleRow pipelines."""
                        wk_t = wkp.tile([P, 8, 256], fp8, tag="wk")
                        nc.sync.dma_start(wk_t, wk_d[gp])
                        wq_t = wqp.tile([P, 8, 256], fp8, tag="wq")
                        nc.sync.dma_start(wq_t, wq_d[gp])
                        wv_t = wvp.tile([P, 8, 256], fp8, tag="wv")
                        nc.sync.dma_start(wv_t, wv_d[gp])
                        KT2 = gpool.tile([P, 2, T], bf16, tag="KT2")
                        QT2 = gpool.tile([P, 2, TQ], bf16, tag="QT2")
                        vaug = vpool.tile([P, 16, 4, P], fp8, tag="vaug")
                        units = []

                        def u_vones():
                            # col 64 is the softmax-denominator ones column;
                            # cols 65:128 pad DR weights to M=128 (their out
                            # rows are never read)
                            nc.gpsimd.memset(vaug[:, :, :, 64:P], 1.0)
                        units.append(u_vones)

                        def mk_k(gi, rc):
                            def u():
                                g = 2 * gp + gi
                                cols = slice(rc * 512, (rc + 1) * 512)
                                psum_k = ps_ab.tile([P, 512], f32, tag="ab")
                                for k in range(4):
                                    nc.tensor.matmul(
                                        psum_k,
                                        wk_t[:, 2 * k:2 * k + 2,
                                             128 * gi:128 * (gi + 1)],
                                        h8c[rc][:, 2 * k:2 * k + 2, :],
                                        start=(k == 0), stop=(k == 3),
                                        perf_mode=DR)
                                nc.vector.tensor_scalar(
                                    KT2[:, gi, cols], psum_k,
                                    1.0 / (WK8 * H8), bk_sb[:, g:g + 1],
                                    op0=AL.mult, op1=AL.add)
                            return u

                        def mk_q(gi, cc):
                            def u():
                                g = 2 * gp + gi
                                cols = slice(cc * 256, (cc + 1) * 256)
                                psum_q = ps_ab.tile([P, 256], f32, tag="ab")
                                for k in range(4):
                                    nc.tensor.matmul(
                                        psum_q,
                                        wq_t[:, 2 * k:2 * k + 2,
                                             128 * gi:128 * (gi + 1)],
                                        h8c[cc][:, 2 * k:2 * k + 2, 0:256],
                                        start=(k == 0), stop=(k == 3),
                                        perf_mode=DR)
                                nc.vector.tensor_scalar(
                                    QT2[:, gi, cols], psum_q,
                                    1.0 / (WQ8 * H8), bq_sb[:, g:g + 1],
                                    op0=AL.mult, op1=AL.add)
                            return u

                        def mk_v(kt):
                            def u():
                                psum_v = ps_ab.tile([P, 4, 64], f32, tag="ab")
                                cc, t = kt // 4, kt % 4
                                for k in range(4):
                                    nc.tensor.matmul(
                                        psum_v,
                                        h8c[cc][:, 2 * k:2 * k + 2,
                                                t * P:(t + 1) * P],
                                        wv_t[:, 2 * k:2 * k + 2, :],
                                        start=(k == 0), stop=(k == 3),
                                        perf_mode=DR)
                                nc.vector.tensor_scalar_mul(
                                    vaug[:, kt, :, 0:64], psum_v,
                                    1.0 / (WV8 * H8))
                            return u

                        for rc in range(4):
                            units.append(mk_k(0, rc))
                            units.append(mk_k(1, rc))
                        for cc in range(4):
                            units.append(mk_q(0, cc))
                            units.append(mk_q(1, cc))
                        for kt in range(16):
                            units.append(mk_v(kt))
                        return (KT2, QT2, vaug), units

                    def emit_gp(gp, tiles, units_next, ustart=0, lag=10):
                        """Emit gp's S/exp/AV stream with a one-batch AV lag,
                        interleaving the next gp's K/Q/V units into the exp
                        latency gaps."""
                        KT2, QT2, vaug = tiles
                        batches = []
                        for pos in range(4):
                            for gi in range(2):
                                for hh in range(2):
                                    for done in range(0, 4 * pos + 4, 4):
                                        batches.append((gi, pos, hh, done))
                        pending = []
                        psum_os = {}
                        den_ps = {}
                        ui = 0

                        def emit_av(batch, pt):
                            gi, pos, hh, done = batch
                            nkt = 4 * pos + 4
                            qc = slice(pos * 256, (pos + 1) * 256)
                            if (gi, pos) not in psum_os:
                                psum_os[(gi, pos)] = ps_o.tile(
                                    [P, 2, 256], f32, tag="o",
                                    name=f"psum_o_{gp}_{gi}_{pos}")
                            if gi not in den_ps:
                                den_ps[gi] = tuple(
                                    dpool.tile([1, TQ], f32r,
                                               tag=f"den{gi}{h2}",
                                               name=f"den_{gp}_{gi}_{h2}")
                                    for h2 in range(2))
                            psum_o = psum_os[(gi, pos)]
                            for t in range(2):
                                kt0 = done + 2 * t
                                nc.tensor.matmul(
                                    psum_o[:, hh, :],
                                    vaug[:, kt0:kt0 + 2, 2 * gi + hh, 0:P],
                                    pt[:, 2 * t:2 * t + 2, :],
                                    start=(kt0 == 0), stop=(kt0 + 2 == nkt),
                                    perf_mode=DR)
                            if done + 4 == nkt and hh == 1:
                                den_p = den_ps[gi]
                                g = 2 * gp + gi
                                bc_sb = dpool.tile(
                                    [P, 2, 256], bf16, tag="bcs",
                                    name=f"bcs_{gp}_{gi}_{pos}")
                                for h2 in range(2):
                                    with nc.allow_low_precision(
                                            reason="f32r has f32 bits"):
                                        nc.vector.reciprocal(
                                            den_p[h2][:, qc],
                                            psum_o[64:65, h2, :])
                                    psum_bc = ps_ab.tile(
                                        [64, 256], f32, tag="ab",
                                        name=f"bc_{gp}_{gi}_{pos}_{h2}")
                                    nc.tensor.matmul(
                                        psum_bc, ones128[:, 0:64],
                                        den_p[h2][:, qc],
                                        start=True, stop=True)
                                    nc.vector.tensor_copy(
                                        bc_sb[0:64, h2, :], psum_bc)
                                for h2 in range(2):
                                    nc.vector.tensor_tensor(
                                        OT8[64 * h2:64 * h2 + 64, g, qc],
                                        psum_o[0:64, h2, :],
                                        bc_sb[0:64, h2, :],
                                        AL.mult)

                        for i, batch in enumerate(batches):
                            gi, pos, hh, done = batch
                            nkt = 4 * pos + 4
                            final = (done + 4 == nkt)
                            qc = slice(pos * 256, (pos + 1) * 256)
                            hb = slice(64 * hh, 64 * hh + 64)
                            psum_s = ps_s.tile([P, 4, 256], f32, tag="s")
                            if final:
                                for bh in range(2):
                                    nc.tensor.matmul(
                                        psum_s[:, 2 * bh:2 * bh + 2, :], ipair,
                                        sbias[:, :, 512 * bh:512 * (bh + 1)],
                                        start=True, stop=False,
                                        perf_mode=DR, skip_group_check=True)
                            for j in range(4):
                                kt = done + j
                                nc.tensor.matmul(
                                    psum_s[:, j, :],
                                    KT2[hb, gi, kt * P:(kt + 1) * P],
                                    QT2[hb, gi, qc],
                                    start=(not final), stop=True,
                                    skip_group_check=True)
                            pt = ptpool.tile([P, 4, 256], fp8, tag="pt")
                            nc.scalar.activation(pt, psum_s, AF.Exp)
                            pending.append((batch, pt))
                            if len(pending) > lag:
                                emit_av(*pending.pop(0))
                            if i + 1 > ustart:
                                target = ((i + 1 - ustart) * len(units_next)
                                          // (len(batches) - ustart))
                            else:
                                target = 0
                            while ui < target:
                                units_next[ui]()
                                ui += 1
                        while pending:
                            emit_av(*pending.pop(0))
                        while ui < len(units_next):
                            units_next[ui]()
                            ui += 1

                    with tc.tile_pool(name="sq_sb", bufs=6) as sqpool, \
                         tc.tile_pool(name="ln_sb", bufs=4) as lnsb, \
                         tc.tile_pool(name="ln_small", bufs=2) as lnsm:
                        for ft in range(8):
                            nc.sync.dma_start(xTc[0][:, ft, :],
                                              xkvt_d[ft][:, 0:512])
                        nc.sync.dma_start(ball, ball_d[:, :])
                        nc.sync.dma_start(sbias, sbias_d[:, :, :])
                        for cc in range(1, 4):
                            for ft in range(8):
                                nc.sync.dma_start(
                                    xTc[cc][:, ft, :],
                                    xkvt_d[ft][:, cc * 512:(cc + 1) * 512])
                        tiles_cur, units_cur = make_units(0)
                        nc.sync.dma_start(wp_t, wp_d[:, :, :])
                        for ft in range(8):
                            nc.sync.dma_start(xqT[:, ft, :], xqres_d[ft])
                        # unit index map: 0=vones, 1+2rc+gi=K, 9+2cc+gi=Q,
                        # 17+kt=V; K/Q/V units for chunk cc depend on h8c[cc]
                        units_cur[0]()
                        cunits = [(cc, [units_cur[1 + 2 * cc],
                                        units_cur[2 + 2 * cc],
                                        units_cur[9 + 2 * cc],
                                        units_cur[10 + 2 * cc]] +
                                   [units_cur[17 + kt]
                                    for kt in range(4 * cc, 4 * cc + 4)])
                                  for cc in range(4)]
                        half = slice(0, 512)
                        # two-deep software pipeline: stats(cc+1) overlaps
                        # finish(cc)'s chain; units run after their chunk
                        rows_q = [ln_stats(xTc[0], half, ps_ab, lnsm, lnsb,
                                           ptag="ab", big_pool=ps_s,
                                           sqp=sqpool)]
                        for cc in range(4):
                            if cc + 1 < 4:
                                rows_q.append(
                                    ln_stats(xTc[cc + 1], half, ps_ab, lnsm,
                                             lnsb, ptag="ab", big_pool=ps_s,
                                             sqp=sqpool))
                            ln_finish(xTc[cc], h8c[cc], half, rows_q[cc],
                                      ps_ab, lnsb, ptag="ab",
                                      pool_split=(cc < 2))
                            if cc < 2:
                                for u in cunits[cc][1]:
                                    u()
                        carry = []
                        for cc in (2, 3):
                            us = cunits[cc][1]
                            carry.extend([us[2], us[3], us[0], us[1]] + us[4:])
                        for gp in range(4):
                            if gp < 3:
                                tiles_next, units_next = make_units(gp + 1)
                            else:
                                tiles_next, units_next = None, []
                            if gp == 0:
                                units_next = carry + units_next
                            emit_gp(gp, tiles_cur, units_next,
                                    ustart=(10 if gp == 0 else 0),
                                    lag=10)
                            tiles_cur = tiles_next

            # ---------------- Phase B: proj + residual + LN2 ----------------
            with tc.tile_pool(name="late_pers", bufs=1) as late:
                x2T = late.tile([P, 8, TQ], bf16)      # post-proj residual
                h2f8 = late.tile([P, 8, TQ], fp8)      # LN2 output (x8, fp8)
                with tc.tile_pool(name="ln2_sb", bufs=4) as lnsb2, \
                     tc.tile_pool(name="ln2_small", bufs=2) as lnsm2, \
                     tc.tile_pool(name="ln2_ps", bufs=1, space="PSUM") as lnps2, \
                     tc.tile_pool(name="w1_sb", bufs=8) as w1pool, \
                     tc.tile_pool(name="w2_sb", bufs=8) as w2pool, \
                     tc.tile_pool(name="relu_sb", bufs=1) as rpool, \
                     tc.tile_pool(name="y_sb", bufs=2) as ypool, \
                     tc.tile_pool(name="ps_f1", bufs=3, space="PSUM") as ps_f1, \
                     tc.tile_pool(name="ps_f2", bufs=2, space="PSUM") as ps_f2:
                    relu1T = rpool.tile([P, 32, TQ], fp8)

                    def proj_unit(of, rc):
                        cols = slice(rc * 512, (rc + 1) * 512)
                        psum_p = ps_f1.tile([P, 512], f32, tag="f1", name=f"pp_{of}_{rc}")
                        nc.tensor.matmul(
                            psum_p, bprow[:, of, :], onesrow,
                            start=True, stop=False,
                            skip_group_check=True)
                        for k in range(4):
                            nc.tensor.matmul(
                                psum_p,
                                wp_t[:, 2 * k:2 * k + 2,
                                     of * P:(of + 1) * P],
                                OT8[:, 2 * k:2 * k + 2, cols],
                                start=False, stop=(k == 3),
                                perf_mode=DR, skip_group_check=True)
                        nc.vector.scalar_tensor_tensor(
                            x2T[:, of, cols], psum_p, 1.0 / WP8,
                            xqT[:, of, cols], op0=AL.mult, op1=AL.add)

                    w1_ts, w2_ts = {}, {}

                    def w1_load(fk, rc):
                        w1_ts[(fk, rc)] = w1pool.tile(
                            [P, 8, P], fp8, tag="w1", name=f"w1_{rc}_{fk}")
                        nc.sync.dma_start(w1_ts[(fk, rc)], w1_d[fk])

                    def w2_load(of, rc):
                        if (of, 0) in w2_ts:
                            w2_ts[(of, rc)] = w2_ts[(of, 0)]
                            return
                        w2_ts[(of, rc)] = w2pool.tile(
                            [P, 32, P], fp8, tag="w2", name=f"w2_{rc}_{of}")
                        nc.sync.dma_start(w2_ts[(of, rc)], w2_d[of])

                    def ffn1_unit(fk, rc):
                        w1_t = w1_ts.pop((fk, rc))
                        cols = slice(rc * 512, (rc + 1) * 512)
                        psum_f = ps_f1.tile([P, 512], f32, tag="f1")
                        for kk in range(4):
                            nc.tensor.matmul(
                                psum_f,
                                w1_t[:, 2 * kk:2 * kk + 2, :],
                                h2f8[:, 2 * kk:2 * kk + 2, cols],
                                start=(kk == 0), stop=(kk == 3),
                                perf_mode=DR)
                        nc.scalar.activation(
                            relu1T[:, fk, cols], psum_f, AF.Relu,
                            bias=b1_sb[:, fk:fk + 1],
                            scale=1.0 / (W18 * H8))

                    def ffn2_unit(of, rc):
                        w2_t = w2_ts[(of, rc)]
                        cols = slice(rc * 512, (rc + 1) * 512)
                        psum_f2 = ps_f2.tile([P, 512], f32, tag="f2")
                        nc.tensor.matmul(
                            psum_f2, b2row[:, of, :], onesrow,
                            start=True, stop=False,
                            skip_group_check=True)
                        for kk in range(16):
                            nc.tensor.matmul(
                                psum_f2,
                                w2_t[:, 2 * kk:2 * kk + 2, :],
                                relu1T[:, 2 * kk:2 * kk + 2, cols],
                                start=False, stop=(kk == 15),
                                perf_mode=DR, skip_group_check=True)
                        y_sb = ypool.tile([P, 512], f32, tag="y")
                        nc.vector.scalar_tensor_tensor(
                            y_sb, psum_f2, 1.0 / W28,
                            x2T[:, of, cols], op0=AL.mult, op1=AL.add)
                        nc.scalar.dma_start(
                            y_d[of * P:(of + 1) * P, cols], y_sb)

                    # column-pipelined tail: proj rc1 overlaps LN2(c0)'s
                    # chain; FFN1 rc0 overlaps LN2(c1); FFN2 rc0 overlaps
                    # FFN1 rc1. Weight DMAs prefetched two units ahead.
                    for of in range(8):
                        w2_load(of, 0)
                    for of in range(8):
                        proj_unit(of, 0)
                    c0 = slice(0, 512)
                    rows0 = ln_stats(x2T, c0, lnps2, lnsm2, lnsb2)
                    for of in range(8):
                        proj_unit(of, 1)
                    ln_finish(x2T, h2f8, c0, rows0, lnps2, lnsb2)
                    c1 = slice(512, 1024)
                    rows1 = ln_stats(x2T, c1, lnps2, lnsm2, lnsb2)

                    sched = [("f1", fk, 0) for fk in range(32)]
                    for fk in range(32):
                        sched.append(("f1", fk, 1))
                        if fk % 4 == 3:
                            sched.append(("f2", fk // 4, 0))
                    sched += [("f2", of, 1) for of in range(8)]
                    loads = {"f1": w1_load, "f2": w2_load}
                    units = {"f1": ffn1_unit, "f2": ffn2_unit}
                    for i in range(6):
                        loads[sched[i][0]](*sched[i][1:])
                    ln1_done = False
                    for i, (kind, a, b) in enumerate(sched):
                        if i + 6 < len(sched):
                            nxt = sched[i + 6]
                            loads[nxt[0]](*nxt[1:])
                        units[kind](a, b)
                        if i == 7 and not ln1_done:
                            ln_finish(x2T, h2f8, c1, rows1, lnps2, lnsb2)
                            ln1_done = True
    _split_sync_waits(nc)
    return nc


_PROGRAM = None


def _get_program():
    global _PROGRAM
    if _PROGRAM is None:
        _PROGRAM = build_program()
    return _PROGRAM


def _host_prep(x, Wk, Wq, Wv, Wproj, bproj, W1, b1, W2, b2, g1, beta1, g2, beta2):
    """Fold LN affines into weights; build per-core shards (host work is
    layout marshalling only -- all input-dependent math runs on device)."""
    import ml_dtypes

    bfl = ml_dtypes.bfloat16
    f8l = ml_dtypes.float8_e4m3
    x = np.asarray(x, np.float32)
    Wq = np.asarray(Wq, np.float32)
    Wk = np.asarray(Wk, np.float32)
    Wv = np.asarray(Wv, np.float32)
    Wproj = np.asarray(Wproj, np.float32)
    W1 = np.asarray(W1, np.float32)
    W2 = np.asarray(W2, np.float32)
    g1 = np.asarray(g1, np.float32)
    beta1 = np.asarray(beta1, np.float32)
    g2 = np.asarray(g2, np.float32)
    beta2 = np.asarray(beta2, np.float32)

    scale = 1.0 / np.sqrt(D)
    Wq_f = (g1[:, None] * Wq) * scale
    bq_f = (beta1 @ Wq) * scale
    Wk_f = g1[:, None] * Wk
    bk_f = beta1 @ Wk
    Wv_f = g1[:, None] * Wv
    bv_f = beta1 @ Wv
    # softmax rows sum to 1 -> the V bias lands as a constant on O; fold it
    # into the proj bias
    bp_f = np.asarray(bproj, np.float32) + bv_f @ Wproj
    W1_f = g2[:, None] * W1
    b1_f = np.asarray(b1, np.float32) + beta2 @ W1

    def tile_in_out(W, n_in, n_out, osz=P, dt=f8l):
        # [in, out] -> [n_out, 128, n_in, osz]
        return np.ascontiguousarray(
            W.reshape(n_in, P, n_out, osz).transpose(2, 1, 0, 3).astype(dt))

    # additive causal bias tiles for the final 4-kt group: slot 0 = tri0,
    # slot 1 = tri1; slots 2,3: -30 for set 0 (beyond its queries), 0 for
    # set 1 (fully-allowed earlier keys in its swapped layout)
    tri0 = (np.arange(256)[None, :] >= np.arange(P)[:, None])
    tri1 = (np.arange(256)[None, :] >= np.arange(P)[:, None] + 128)
    sbias_sets = []
    for s in range(2):
        m = np.zeros((P, 2, 4, 256), np.float32)
        m[:, 0, 0, :] = np.where(tri0, 0.0, -30.0)
        m[:, 0, 1, :] = np.where(tri1, 0.0, -30.0)
        if s == 0:
            m[:, 0, 2, :] = -30.0
            m[:, 0, 3, :] = -30.0
        sbias_sets.append(m.reshape(P, 2, 1024).astype(f8l))

    ipair = np.zeros((P, 2, P), np.float32)
    ipair[:, 0, :] = np.eye(P)
    sel2 = np.zeros((2, P), np.float32)
    sel2[0, 0:64] = 1.0
    sel2[1, 64:128] = 1.0

    common = {
        "wq": tile_in_out(WQ8 * Wq_f, 8, 4, osz=256),
        "wk": tile_in_out(WK8 * Wk_f, 8, 4, osz=256),
        "wv": tile_in_out(WV8 * Wv_f, 8, 4, osz=256),
        "wp": np.ascontiguousarray(
            (WP8 * Wproj).reshape(8, P, C).transpose(1, 0, 2).astype(f8l)),
        "w1": tile_in_out(W18 * W1_f, 8, 32),
        "w2": tile_in_out(W28 * W2, 32, 8),
        "ball": np.ascontiguousarray(np.concatenate([
            bq_f.reshape(8, P).T, bk_f.reshape(8, P).T,
            b1_f.reshape(32, P).T], axis=1)),
        "bprow": np.ascontiguousarray(
            (WP8 * bp_f).reshape(1, 8, P).astype(bfl)),
        "b2row": np.ascontiguousarray(
            (W28 * np.asarray(b2, np.float32)).reshape(1, 8, P).astype(bfl)),
        "ipair": np.ascontiguousarray(ipair.astype(f8l)),
        "sel2": np.ascontiguousarray(sel2),
        "ones128": np.ones((1, P), np.float32),
        "eights128": np.full((1, P), 8.0, np.float32),
        "onesrow": np.ones((1, 512), bfl),
        "onescolbf": np.ones((P, 1), bfl),
    }

    in_maps = []
    row_maps = []
    for core in range(8):
        b, s = core // 2, core % 2
        rows = np.concatenate(
            [np.arange(512 * i + 256 * s, 512 * i + 256 * s + 256)
             for i in range(4)])
        row_maps.append((b, rows))
        xb = x[b]
        if s == 1:
            # swap the 256-halves of every 512-token chunk so this core's
            # own (odd-block) queries sit at chunk columns [0:256)
            perm = np.concatenate(
                [np.concatenate([np.arange(512 * i + 256, 512 * i + 512),
                                 np.arange(512 * i, 512 * i + 256)])
                 for i in range(4)])
            xkv = xb[perm]
        else:
            xkv = xb
        xq = np.ascontiguousarray(xb[rows])
        in_maps.append({
            "xkvt": np.ascontiguousarray(xkv.T.reshape(8, P, T).astype(bfl)),
            "xqres": np.ascontiguousarray(xq.T.reshape(8, P, TQ).astype(bfl)),
            "sbias": sbias_sets[s],
            **common,
        })
    return in_maps, row_maps


def kernel(**inputs):
    nc = _get_program()
    in_maps, row_maps = _host_prep(**inputs)
    res = run_bass_kernel_spmd(nc, in_maps, core_ids=list(range(8)))
    out = np.empty((B, T, C), np.float32)
    for core in range(8):
        b, rows = row_maps[core]
        out[b][rows] = res.results[core]["y"].T
    return out


if __name__ == "__main__":
    from concourse.timeline_sim import TimelineSim
    print("sim ns:", TimelineSim(build_program(), trace=False).simulate())


# revision 79
# speedup vs baseline: 1.0018x; 1.0018x over previous
"""Trainium2 Bass kernel for one dense transformer block.

Full (unsharded) IO: x [4, 2048, 1024] -> out [4, 2048, 1024].
Sharding: 8 cores = 4 batches x 2 query sets. Each core owns one batch's K/V
(2048 rows) and 1024 query rows. Set 0 takes even 256-row blocks {0,2,4,6},
set 1 odd blocks {1,3,5,7}. Set-1 cores store their keys with the two
256-halves of every 512-token chunk swapped, so every core's own query rows
sit at columns [0:256) of each 512-chunk of its key layout -- the Q
projection reads the LN output directly (no separate q-side LayerNorm), and
the causal boundary tiles sit at the same slot positions on every core
(masks are per-core data; the instruction stream is identical).

Numerics: fp8e4 DoubleRow matmuls (0.5 cyc/col, 256-deep contraction) for
Q/K/V/AV/proj/FFN; S stays bf16. Causal masking is additive: a -30 bias is
preloaded into the S psum via one fp8-DR identity matmul, so exp() output
is written as fp8 directly and feeds DoubleRow AV matmuls. The softmax
denominator rides as a ones-column in the V tiles (padded to 128-wide DR
weights; the pad rows' outputs are never read); normalization is a DVE
reciprocal + PE broadcast + DVE multiply per (head, query-block). proj/FFN2
biases are added via rank-1 bf16 matmuls into the psum. The post-attention
tail (proj -> LN2 -> FFN) is column-pipelined so LN2 chains overlap proj/FFN
matmuls.
"""

import sys

sys.path.insert(0, "/opt/trn_rl_repo")

import numpy as np

import concourse.bass as bass
import concourse.mybir as mybir
import concourse.tile as tile
from concourse.bass_utils import run_bass_kernel_spmd

f32 = mybir.dt.float32
f32r = mybir.dt.float32r
bf16 = mybir.dt.bfloat16
fp8 = mybir.dt.float8e4
AL = mybir.AluOpType
AF = mybir.ActivationFunctionType
DR = mybir.MatmulPerfMode.DoubleRow

B, T, C = 4, 2048, 1024
H, D = 16, 64
F = 4 * C
P = 128
TQ = 1024            # query rows per core
LN_EPS = 1e-5
H8 = 8.0             # LN-output fp8 scale
WK8, WQ8, WV8 = 64.0, 512.0, 64.0
WP8, W18, W28 = 64.0, 16.0, 32.0


def _split_sync_waits(nc):
    """This container's walrus supports one sync-wait per instruction; Tile
    emits up to ~3. Hoist extras onto NoOps inserted before the owner."""
    ctr = 0
    for fn in nc.m.functions:
        for bb in fn.blocks:
            out, changed = [], False
            for ins in bb.instructions:
                si = ins.sync_info
                waits = list(si.on_wait) if si is not None and si.on_wait else []
                if len(waits) > 1:
                    changed = True
                    for w in waits[:-1]:
                        ctr += 1
                        nop = mybir.InstNoOp(name=f"waitsplit_{ctr}", ins=[], outs=[])
                        nop.engine = ins.engine
                        nop.sync_info = mybir.SyncInfo(on_wait=[w], on_update=[])
                        out.append(nop)
                        nc.register_instruction(nop, overwrite=True)
                    ins.sync_info = mybir.SyncInfo(
                        on_wait=[waits[-1]], on_update=list(si.on_update or [])
                    )
                out.append(ins)
            if changed:
                bb.instructions = out


def build_program():
    nc = bass.Bass()
    xkvt_d = nc.dram_tensor("xkvt", [8, P, T], bf16, kind="ExternalInput")
    xqres_d = nc.dram_tensor("xqres", [8, P, TQ], bf16, kind="ExternalInput")
    sbias_d = nc.dram_tensor("sbias", [P, 2, 1024], fp8, kind="ExternalInput")
    ipair_d = nc.dram_tensor("ipair", [P, 2, P], fp8, kind="ExternalInput")
    ones128_d = nc.dram_tensor("ones128", [1, P], f32r, kind="ExternalInput")
    eights128_d = nc.dram_tensor("eights128", [1, P], f32r, kind="ExternalInput")
    onescol_bf_d = nc.dram_tensor("onescolbf", [P, 1], bf16, kind="ExternalInput")
    sel2_d = nc.dram_tensor("sel2", [2, P], f32r, kind="ExternalInput")
    onesrow_d = nc.dram_tensor("onesrow", [1, 512], bf16, kind="ExternalInput")
    bprow_d = nc.dram_tensor("bprow", [1, 8, P], bf16, kind="ExternalInput")
    b2row_d = nc.dram_tensor("b2row", [1, 8, P], bf16, kind="ExternalInput")
    wq_d = nc.dram_tensor("wq", [4, P, 8, 256], fp8, kind="ExternalInput")
    wk_d = nc.dram_tensor("wk", [4, P, 8, 256], fp8, kind="ExternalInput")
    wv_d = nc.dram_tensor("wv", [4, P, 8, 256], fp8, kind="ExternalInput")
    wp_d = nc.dram_tensor("wp", [P, 8, C], fp8, kind="ExternalInput")
    w1_d = nc.dram_tensor("w1", [32, P, 8, P], fp8, kind="ExternalInput")
    w2_d = nc.dram_tensor("w2", [8, P, 32, P], fp8, kind="ExternalInput")
    ball_d = nc.dram_tensor("ball", [P, 48], f32, kind="ExternalInput")
    y_d = nc.dram_tensor("y", [C, TQ], f32, kind="ExternalOutput")

    with tile.TileContext(nc) as tc:
        with tc.tile_pool(name="consts", bufs=1) as cpool, \
             tc.tile_pool(name="pers", bufs=1) as pers, \
             tc.tile_pool(name="proj_in", bufs=1) as prpool:
            ones128 = cpool.tile([1, P], f32r)
            nc.sync.dma_start(ones128, ones128_d[:, :])
            eights128 = cpool.tile([1, P], f32r)
            nc.sync.dma_start(eights128, eights128_d[:, :])
            onescol_bf = cpool.tile([P, 1], bf16)
            nc.sync.dma_start(onescol_bf, onescol_bf_d[:, :])
            sel2 = cpool.tile([2, P], f32r)
            nc.sync.dma_start(sel2, sel2_d[:, :])
            onesrow = cpool.tile([1, 512], bf16)
            nc.sync.dma_start(onesrow, onesrow_d[:, :])
            bprow = cpool.tile([1, 8, P], bf16)
            nc.sync.dma_start(bprow, bprow_d[:, :, :])
            b2row = cpool.tile([1, 8, P], bf16)
            nc.sync.dma_start(b2row, b2row_d[:, :, :])
            ipair = cpool.tile([P, 2, P], fp8)
            nc.sync.dma_start(ipair, ipair_d[:, :, :])
            sbias = cpool.tile([P, 2, 1024], fp8)
            eps1 = cpool.tile([1, 1], f32)
            nc.vector.memset(eps1, LN_EPS)
            ball = cpool.tile([P, 48], f32)
            bq_sb = ball[:, 0:8]
            bk_sb = ball[:, 8:16]
            b1_sb = ball[:, 16:48]

            OT8 = pers.tile([P, 8, TQ], fp8)       # attn out / den, fp8

            def ln_stats(src, cols, lnps, lnsm, lnsb, ptag=None, big_pool=None,
                         sq_dve=False, sqp=None):
                """Sum / sum-of-squares ones-matmuls over one 512-col chunk
                of a [P, 8, N] bf16 tensor, then the small chain down to the
                rstd / mean*rstd row vectors."""
                if big_pool is not None:
                    psum_smt = big_pool.tile([P, 4, 256], f32, tag="s",
                                             name="psum_smt")
                    psum_sqt = big_pool.tile([P, 4, 256], f32, tag="s",
                                             name="psum_sqt")
                    psum_sm = psum_smt[0:1, 0:2, :]
                    psum_sq = psum_sqt[0:1, 0:2, :]
                else:
                    psum_sm = lnps.tile([1, 512], f32, tag=ptag or "sm",
                                        name="psum_sm")
                    psum_sq = lnps.tile([1, 512], f32, tag=ptag or "sq",
                                        name="psum_sq")
                for ft in range(8):
                    sq = (sqp or lnsb).tile([P, 512], bf16, tag="sq")
                    if sq_dve:
                        nc.vector.tensor_tensor(
                            sq, src[:, ft, cols], src[:, ft, cols], AL.mult)
                    else:
                        nc.scalar.activation(sq, src[:, ft, cols], AF.Square)
                    nc.tensor.matmul(psum_sm, onescol_bf, src[:, ft, cols],
                                     start=(ft == 0), stop=(ft == 7))
                    nc.tensor.matmul(psum_sq, onescol_bf, sq,
                                     start=(ft == 0), stop=(ft == 7))
                mean = lnsm.tile([1, 512], f32, tag="mean")
                nc.vector.tensor_scalar_mul(mean, psum_sm, 1.0 / C)
                msq = lnsm.tile([1, 512], f32, tag="msq")
                nc.vector.tensor_scalar_mul(msq, psum_sq, 1.0 / C)
                var = lnsm.tile([1, 512], f32, tag="var")
                nc.vector.tensor_tensor(var, mean, mean, AL.mult)
                nc.vector.tensor_tensor(var, msq, var, AL.subtract)
                rrow = lnsm.tile([1, 512], f32r, tag="rrow")
                nc.scalar.activation(rrow, var, AF.Sqrt, bias=eps1, scale=1.0)
                with nc.allow_low_precision(reason="f32r has f32 bits"):
                    nc.vector.reciprocal(rrow, rrow)
                mrow = lnsm.tile([1, 512], f32r, tag="mrow")
                nc.vector.tensor_tensor(mrow, mean, rrow, AL.mult)
                return rrow, mrow

            def ln_finish(src, dst, cols, rows, lnps, lnsb, ptag=None,
                          pool_split=True):
                """Broadcast the (x8) row vectors along partitions and apply;
                dst is fp8 (x8-scaled LN output)."""
                rrow, mrow = rows
                psum_r = lnps.tile([P, 512], f32, tag=ptag or "bc",
                                   name="psum_r")
                nc.tensor.matmul(psum_r, eights128, rrow, start=True, stop=True)
                psum_m = lnps.tile([P, 512], f32, tag=ptag or "bc",
                                   name="psum_m")
                nc.tensor.matmul(psum_m, eights128, mrow, start=True, stop=True)
                rbc = lnsb.tile([P, 512], bf16, tag="rbc")
                nc.scalar.copy(rbc, psum_r)
                mbc = lnsb.tile([P, 512], bf16, tag="mbc")
                nc.scalar.copy(mbc, psum_m)
                for ft in range(8):
                    tmp = lnsb.tile([P, 512], bf16, tag="tmp")
                    k = 2 if pool_split else 4
                    eng = nc.gpsimd if ft % k == 0 else nc.vector
                    eng.tensor_tensor(
                        tmp, src[:, ft, cols], rbc, AL.mult)
                    eng2 = nc.gpsimd if ft % k == k - 1 else nc.vector
                    eng2.tensor_tensor(
                        dst[:, ft, cols], tmp, mbc, AL.subtract)

            # ------------- Phase 0: load xT; LN1 -> h8 (fp8, x8) -------------
            with tc.tile_pool(name="xt_sb", bufs=1) as xtpool:
                wp_t = prpool.tile([P, 8, C], fp8)
                xqT = prpool.tile([P, 8, TQ], bf16)
                xTc = [xtpool.tile([P, 8, 512], bf16, name=f"xtc{cc}")
                       for cc in range(4)]
                h8c = [prpool.tile([P, 8, 512], fp8, name=f"h8c{cc}")
                       for cc in range(4)]

                # ---------------- Phase A: attention ----------------
                with tc.tile_pool(name="wk_p", bufs=2) as wkp, \
                     tc.tile_pool(name="wq_p", bufs=2) as wqp, \
                     tc.tile_pool(name="wv_p", bufs=1) as wvp, \
                     tc.tile_pool(name="kq_big", bufs=2) as gpool, \
                     tc.tile_pool(name="v_sb", bufs=2) as vpool, \
                     tc.tile_pool(name="pt_sb", bufs=12) as ptpool, \
                     tc.tile_pool(name="den_sb", bufs=1) as dpool, \
                     tc.tile_pool(name="ps_ab", bufs=2, space="PSUM") as ps_ab, \
                     tc.tile_pool(name="ps_s", bufs=2, space="PSUM") as ps_s, \
                     tc.tile_pool(name="ps_o", bufs=2, space="PSUM") as ps_o:

                    def make_units(gp):
                        """Prefetch gp's weights now; return (tiles, unit
                        closures) emitting gp's K/Q/V DoubleRow pipelines."""
                        wk_t = wkp.tile([P, 8, 256], fp8, tag="wk")
                        nc.sync.dma_start(wk_t, wk_d[gp])
                        wq_t = wqp.tile([P, 8, 256], fp8, tag="wq")
                        nc.sync.dma_start(wq_t, wq_d[gp])
                        wv_t = wvp.tile([P, 8, 256], fp8, tag="wv")
                        nc.sync.dma_start(wv_t, wv_d[gp])
                        KT2 = gpool.tile([P, 2, T], bf16, tag="KT2")
                        QT2 = gpool.tile([P, 2, TQ], bf16, tag="QT2")
                        vaug = vpool.tile([P, 16, 4, P], fp8, tag="vaug")
                        units = []

                        def u_vones():
                            # col 64 is the softmax-denominator ones column;
                            # cols 65:128 pad DR weights to M=128 (their out
                            # rows are never read)
                            nc.gpsimd.memset(vaug[:, :, :, 64:P], 1.0)
                        units.append(u_vones)

                        def mk_k(gi, rc):
                            def u():
                                g = 2 * gp + gi
                                cols = slice(rc * 512, (rc + 1) * 512)
                                psum_k = ps_ab.tile([P, 512], f32, tag="ab")
                                for k in range(4):
                                    nc.tensor.matmul(
                                        psum_k,
                                        wk_t[:, 2 * k:2 * k + 2,
                                             128 * gi:128 * (gi + 1)],
                                        h8c[rc][:, 2 * k:2 * k + 2, :],
                                        start=(k == 0), stop=(k == 3),
                                        perf_mode=DR)
                                nc.vector.tensor_scalar(
                                    KT2[:, gi, cols], psum_k,
                                    1.0 / (WK8 * H8), bk_sb[:, g:g + 1],
                                    op0=AL.mult, op1=AL.add)
                            return u

                        def mk_q(gi, cc):
                            def u():
                                g = 2 * gp + gi
                                cols = slice(cc * 256, (cc + 1) * 256)
                                psum_q = ps_ab.tile([P, 256], f32, tag="ab")
                                for k in range(4):
                                    nc.tensor.matmul(
                                        psum_q,
                                        wq_t[:, 2 * k:2 * k + 2,
                                             128 * gi:128 * (gi + 1)],
                                        h8c[cc][:, 2 * k:2 * k + 2, 0:256],
                                        start=(k == 0), stop=(k == 3),
                                        perf_mode=DR)
                                nc.vector.tensor_scalar(
                                    QT2[:, gi, cols], psum_q,
                                    1.0 / (WQ8 * H8), bq_sb[:, g:g + 1],
                                    op0=AL.mult, op1=AL.add)
                            return u

                        def mk_v(kt):
                            def u():
                                psum_v = ps_ab.tile([P, 4, 64], f32, tag="ab")
                                cc, t = kt // 4, kt % 4
                                for k in range(4):
                                    nc.tensor.matmul(
                                        psum_v,
                                        h8c[cc][:, 2 * k:2 * k + 2,
                                                t * P:(t + 1) * P],
                                        wv_t[:, 2 * k:2 * k + 2, :],
                                        start=(k == 0), stop=(k == 3),
                                        perf_mode=DR)
                                nc.vector.tensor_scalar_mul(
                                    vaug[:, kt, :, 0:64], psum_v,
                                    1.0 / (WV8 * H8))
                            return u

                        for rc in range(4):
                            units.append(mk_k(0, rc))
                            units.append(mk_k(1, rc))
                        for cc in range(4):
                            units.append(mk_q(0, cc))
                            units.append(mk_q(1, cc))
                        for kt in range(16):
                            units.append(mk_v(kt))
                        return (KT2, QT2, vaug), units

                    def emit_gp(gp, tiles, units_next, ustart=0, lag=10):
                        """Emit gp's S/exp/AV stream with a one-batch AV lag,
                        interleaving the next gp's K/Q/V units into the exp
                        latency gaps."""
                        KT2, QT2, vaug = tiles
                        batches = []
                        for pos in range(4):
                            for gi in range(2):
                                for hh in range(2):
                                    for done in range(0, 4 * pos + 4, 4):
                                        batches.append((gi, pos, hh, done))
                        pending = []
                        psum_os = {}
                        den_ps = {}
                        ui = 0

                        def emit_av(batch, pt):
                            gi, pos, hh, done = batch
                            nkt = 4 * pos + 4
                            qc = slice(pos * 256, (pos + 1) * 256)
                            if (gi, pos) not in psum_os:
                                psum_os[(gi, pos)] = ps_o.tile(
                                    [P, 2, 256], f32, tag="o",
                                    name=f"psum_o_{gp}_{gi}_{pos}")
                            if gi not in den_ps:
                                den_ps[gi] = tuple(
                                    dpool.tile([1, TQ], f32r,
                                               tag=f"den{gi}{h2}",
                                               name=f"den_{gp}_{gi}_{h2}")
                                    for h2 in range(2))
                            psum_o = psum_os[(gi, pos)]
                            for t in range(2):
                                kt0 = done + 2 * t
                                nc.tensor.matmul(
                                    psum_o[:, hh, :],
                                    vaug[:, kt0:kt0 + 2, 2 * gi + hh, 0:P],
                                    pt[:, 2 * t:2 * t + 2, :],
                                    start=(kt0 == 0), stop=(kt0 + 2 == nkt),
                                    perf_mode=DR)
                            if done + 4 == nkt and hh == 1:
                                den_p = den_ps[gi]
                                g = 2 * gp + gi
                                bc_sb = dpool.tile(
                                    [P, 2, 256], bf16, tag="bcs",
                                    name=f"bcs_{gp}_{gi}_{pos}")
                                for h2 in range(2):
                                    with nc.allow_low_precision(
                                            reason="f32r has f32 bits"):
                                        nc.vector.reciprocal(
                                            den_p[h2][:, qc],
                                            psum_o[64:65, h2, :])
                                    psum_bc = ps_ab.tile(
                                        [64, 256], f32, tag="ab",
                                        name=f"bc_{gp}_{gi}_{pos}_{h2}")
                                    nc.tensor.matmul(
                                        psum_bc, ones128[:, 0:64],
                                        den_p[h2][:, qc],
                                        start=True, stop=True)
                                    nc.vector.tensor_copy(
                                        bc_sb[0:64, h2, :], psum_bc)
                                for h2 in range(2):
                                    nc.vector.tensor_tensor(
                                        OT8[64 * h2:64 * h2 + 64, g, qc],
                                        psum_o[0:64, h2, :],
                                        bc_sb[0:64, h2, :],
                                        AL.mult)

                        for i, batch in enumerate(batches):
                            gi, pos, hh, done = batch
                            nkt = 4 * pos + 4
                            final = (done + 4 == nkt)
                            qc = slice(pos * 256, (pos + 1) * 256)
                            hb = slice(64 * hh, 64 * hh + 64)
                            psum_s = ps_s.tile([P, 4, 256], f32, tag="s")
                            if final:
                                for bh in range(2):
                                    nc.tensor.matmul(
                                        psum_s[:, 2 * bh:2 * bh + 2, :], ipair,
                                        sbias[:, :, 512 * bh:512 * (bh + 1)],
                                        start=True, stop=False,
                                        perf_mode=DR, skip_group_check=True)
                            for j in range(4):
                                kt = done + j
                                nc.tensor.matmul(
                                    psum_s[:, j, :],
                                    KT2[hb, gi, kt * P:(kt + 1) * P],
                                    QT2[hb, gi, qc],
                                    start=(not final), stop=True,
                                    skip_group_check=True)
                            pt = ptpool.tile([P, 4, 256], fp8, tag="pt")
                            nc.scalar.activation(pt, psum_s, AF.Exp)
                            pending.append((batch, pt))
                            if len(pending) > lag:
                                emit_av(*pending.pop(0))
                            if i + 1 > ustart:
                                target = ((i + 1 - ustart) * len(units_next)
                                          // (len(batches) - ustart))
                            else:
                                target = 0
                            while ui < target:
                                units_next[ui]()
                                ui += 1
                        while pending:
                            emit_av(*pending.pop(0))
                        while ui < len(units_next):
                            units_next[ui]()
                            ui += 1

                    with tc.tile_pool(name="sq_sb", bufs=6) as sqpool, \
                         tc.tile_pool(name="ln_sb", bufs=4) as lnsb, \
                         tc.tile_pool(name="ln_small", bufs=2) as lnsm:
                        for ft in range(8):
                            nc.sync.dma_start(xTc[0][:, ft, :],
                                              xkvt_d[ft][:, 0:512])
                        nc.sync.dma_start(ball, ball_d[:, :])
                        nc.sync.dma_start(sbias, sbias_d[:, :, :])
                        for cc in range(1, 4):
                            for ft in range(8):
                                nc.sync.dma_start(
                                    xTc[cc][:, ft, :],
                                    xkvt_d[ft][:, cc * 512:(cc + 1) * 512])
                        tiles_cur, units_cur = make_units(0)
                        # unit index map: 0=vones, 1+2rc+gi=K, 9+2cc+gi=Q,
                        # 17+kt=V; K/Q/V units for chunk cc depend on h8c[cc]
                        units_cur[0]()
                        cunits = [(cc, [units_cur[1 + 2 * cc],
                                        units_cur[2 + 2 * cc],
                                        units_cur[9 + 2 * cc],
                                        units_cur[10 + 2 * cc]] +
                                   [units_cur[17 + kt]
                                    for kt in range(4 * cc, 4 * cc + 4)])
                                  for cc in range(4)]
                        half = slice(0, 512)
                        # two-deep software pipeline: stats(cc+1) overlaps
                        # finish(cc)'s chain; units run after their chunk
                        rows_q = [ln_stats(xTc[0], half, ps_ab, lnsm, lnsb,
                                           ptag="ab", big_pool=ps_s,
                                           sqp=sqpool)]
                        for cc in range(4):
                            if cc + 1 < 4:
                                rows_q.append(
                                    ln_stats(xTc[cc + 1], half, ps_ab, lnsm,
                                             lnsb, ptag="ab", big_pool=ps_s,
                                             sqp=sqpool))
                            ln_finish(xTc[cc], h8c[cc], half, rows_q[cc],
                                      ps_ab, lnsb, ptag="ab",
                                      pool_split=(cc < 2))
                            if cc < 2:
                                for u in cunits[cc][1]:
                                    u()
                        carry = []
                        for cc in (2, 3):
                            us = cunits[cc][1]
                            carry.extend([us[2], us[3], us[0], us[1]] + us[4:])
                        for gp in range(4):
                            if gp == 1:
                                # proj inputs: DMA is idle once phase 0's
                                # xkvt/weight burst drains
                                nc.sync.dma_start(wp_t, wp_d[:, :, :])
                                for ft in range(8):
                                    nc.sync.dma_start(xqT[:, ft, :],
                                                      xqres_d[ft])
                            if gp < 3:
                                tiles_next, units_next = make_units(gp + 1)
                            else:
                                tiles_next, units_next = None, []
                            if gp == 0:
                                units_next = carry + units_next
                            emit_gp(gp, tiles_cur, units_next,
                                    ustart=(10 if gp == 0 else 0),
                                    lag=10)
                            tiles_cur = tiles_next

            # ---------------- Phase B: proj + residual + LN2 ----------------
            with tc.tile_pool(name="late_pers", bufs=1) as late:
                x2T = late.tile([P, 8, TQ], bf16)      # post-proj residual
                h2f8 = late.tile([P, 8, TQ], fp8)      # LN2 output (x8, fp8)
                with tc.tile_pool(name="ln2_sb", bufs=5) as lnsb2, \
                     tc.tile_pool(name="ln2_small", bufs=2) as lnsm2, \
                     tc.tile_pool(name="ln2_ps", bufs=1, space="PSUM") as lnps2, \
                     tc.tile_pool(name="w1_sb", bufs=8) as w1pool, \
                     tc.tile_pool(name="w2_sb", bufs=8) as w2pool, \
                     tc.tile_pool(name="relu_sb", bufs=1) as rpool, \
                     tc.tile_pool(name="y_sb", bufs=4) as ypool, \
                     tc.tile_pool(name="ps_f1", bufs=3, space="PSUM") as ps_f1, \
                     tc.tile_pool(name="ps_f2", bufs=2, space="PSUM") as ps_f2:
                    relu1T = rpool.tile([P, 32, TQ], fp8)

                    def proj_unit(of, rc):
                        cols = slice(rc * 512, (rc + 1) * 512)
                        psum_p = ps_f1.tile([P, 512], f32, tag="f1", name=f"pp_{of}_{rc}")
                        nc.tensor.matmul(
                            psum_p, bprow[:, of, :], onesrow,
                            start=True, stop=False,
                            skip_group_check=True)
                        for k in range(4):
                            nc.tensor.matmul(
                                psum_p,
                                wp_t[:, 2 * k:2 * k + 2,
                                     of * P:(of + 1) * P],
                                OT8[:, 2 * k:2 * k + 2, cols],
                                start=False, stop=(k == 3),
                                perf_mode=DR, skip_group_check=True)
                        nc.vector.scalar_tensor_tensor(
                            x2T[:, of, cols], psum_p, 1.0 / WP8,
                            xqT[:, of, cols], op0=AL.mult, op1=AL.add)

                    w1_ts, w2_ts = {}, {}

                    def w1_load(fk, rc):
                        w1_ts[(fk, rc)] = w1pool.tile(
                            [P, 8, P], fp8, tag="w1", name=f"w1_{rc}_{fk}")
                        nc.sync.dma_start(w1_ts[(fk, rc)], w1_d[fk])

                    def w2_load(of, rc):
                        if (of, 0) in w2_ts:
                            w2_ts[(of, rc)] = w2_ts[(of, 0)]
                            return
                        w2_ts[(of, rc)] = w2pool.tile(
                            [P, 32, P], fp8, tag="w2", name=f"w2_{rc}_{of}")
                        nc.sync.dma_start(w2_ts[(of, rc)], w2_d[of])

                    def ffn1_unit(fk, rc):
                        w1_t = w1_ts.pop((fk, rc))
                        cols = slice(rc * 512, (rc + 1) * 512)
                        psum_f = ps_f1.tile([P, 512], f32, tag="f1")
                        for kk in range(4):
                            nc.tensor.matmul(
                                psum_f,
                                w1_t[:, 2 * kk:2 * kk + 2, :],
                                h2f8[:, 2 * kk:2 * kk + 2, cols],
                                start=(kk == 0), stop=(kk == 3),
                                perf_mode=DR)
                        nc.scalar.activation(
                            relu1T[:, fk, cols], psum_f, AF.Relu,
                            bias=b1_sb[:, fk:fk + 1],
                            scale=1.0 / (W18 * H8))

                    def ffn2_unit(of, rc):
                        w2_t = w2_ts[(of, rc)]
                        cols = slice(rc * 512, (rc + 1) * 512)
                        psum_f2 = ps_f2.tile([P, 512], f32, tag="f2")
                        nc.tensor.matmul(
                            psum_f2, b2row[:, of, :], onesrow,
                            start=True, stop=False,
                            skip_group_check=True)
                        for kk in range(16):
                            nc.tensor.matmul(
                                psum_f2,
                                w2_t[:, 2 * kk:2 * kk + 2, :],
                                relu1T[:, 2 * kk:2 * kk + 2, cols],
                                start=False, stop=(kk == 15),
                                perf_mode=DR, skip_group_check=True)
                        y_sb = ypool.tile([P, 512], f32, tag="y")
                        nc.vector.scalar_tensor_tensor(
                            y_sb, psum_f2, 1.0 / W28,
                            x2T[:, of, cols], op0=AL.mult, op1=AL.add)
                        nc.scalar.dma_start(
                            y_d[of * P:(of + 1) * P, cols], y_sb)

                    # column-pipelined tail: proj rc1 overlaps LN2(c0)'s
                    # chain; FFN1 rc0 overlaps LN2(c1); FFN2 rc0 overlaps
                    # FFN1 rc1. Weight DMAs prefetched two units ahead.
                    for of in range(8):
                        w2_load(of, 0)
                    for of in range(8):
                        proj_unit(of, 0)
                    c0 = slice(0, 512)
                    rows0 = ln_stats(x2T, c0, lnps2, lnsm2, lnsb2)
                    for of in range(8):
                        proj_unit(of, 1)
                    ln_finish(x2T, h2f8, c0, rows0, lnps2, lnsb2)
                    c1 = slice(512, 1024)
                    rows1 = ln_stats(x2T, c1, lnps2, lnsm2, lnsb2)

                    sched = [("f1", fk, 0) for fk in range(32)]
                    for fk in range(32):
                        sched.append(("f1", fk, 1))
                        if fk % 4 == 3:
                            sched.append(("f2", fk // 4, 0))
                    sched += [("f2", of, 1) for of in range(8)]
                    loads = {"f1": w1_load, "f2": w2_load}
                    units = {"f1": ffn1_unit, "f2": ffn2_unit}
                    for i in range(6):
                        loads[sched[i][0]](*sched[i][1:])
                    ln1_done = False
                    for i, (kind, a, b) in enumerate(sched):
                        if i + 6 < len(sched):
                            nxt = sched[i + 6]
                            loads[nxt[0]](*nxt[1:])
                        units[kind](a, b)
                        if i == 7 and not ln1_done:
                            ln_finish(x2T, h2f8, c1, rows1, lnps2, lnsb2)
                            ln1_done = True
    _split_sync_waits(nc)
    return nc


_PROGRAM = None


def _get_program():
    global _PROGRAM
    if _PROGRAM is None:
        _PROGRAM = build_program()
    return _PROGRAM


def _host_prep(x, Wk, Wq, Wv, Wproj, bproj, W1, b1, W2, b2, g1, beta1, g2, beta2):
    """Fold LN affines into weights; build per-core shards (host work is
    layout marshalling only -- all input-dependent math runs on device)."""
    import ml_dtypes

    bfl = ml_dtypes.bfloat16
    f8l = ml_dtypes.float8_e4m3
    x = np.asarray(x, np.float32)
    Wq = np.asarray(Wq, np.float32)
    Wk = np.asarray(Wk, np.float32)
    Wv = np.asarray(Wv, np.float32)
    Wproj = np.asarray(Wproj, np.float32)
    W1 = np.asarray(W1, np.float32)
    W2 = np.asarray(W2, np.float32)
    g1 = np.asarray(g1, np.float32)
    beta1 = np.asarray(beta1, np.float32)
    g2 = np.asarray(g2, np.float32)
    beta2 = np.asarray(beta2, np.float32)

    scale = 1.0 / np.sqrt(D)
    Wq_f = (g1[:, None] * Wq) * scale
    bq_f = (beta1 @ Wq) * scale
    Wk_f = g1[:, None] * Wk
    bk_f = beta1 @ Wk
    Wv_f = g1[:, None] * Wv
    bv_f = beta1 @ Wv
    # softmax rows sum to 1 -> the V bias lands as a constant on O; fold it
    # into the proj bias
    bp_f = np.asarray(bproj, np.float32) + bv_f @ Wproj
    W1_f = g2[:, None] * W1
    b1_f = np.asarray(b1, np.float32) + beta2 @ W1

    def tile_in_out(W, n_in, n_out, osz=P, dt=f8l):
        # [in, out] -> [n_out, 128, n_in, osz]
        return np.ascontiguousarray(
            W.reshape(n_in, P, n_out, osz).transpose(2, 1, 0, 3).astype(dt))

    # additive causal bias tiles for the final 4-kt group: slot 0 = tri0,
    # slot 1 = tri1; slots 2,3: -30 for set 0 (beyond its queries), 0 for
    # set 1 (fully-allowed earlier keys in its swapped layout)
    tri0 = (np.arange(256)[None, :] >= np.arange(P)[:, None])
    tri1 = (np.arange(256)[None, :] >= np.arange(P)[:, None] + 128)
    sbias_sets = []
    for s in range(2):
        m = np.zeros((P, 2, 4, 256), np.float32)
        m[:, 0, 0, :] = np.where(tri0, 0.0, -30.0)
        m[:, 0, 1, :] = np.where(tri1, 0.0, -30.0)
        if s == 0:
            m[:, 0, 2, :] = -30.0
            m[:, 0, 3, :] = -30.0
        sbias_sets.append(m.reshape(P, 2, 1024).astype(f8l))

    ipair = np.zeros((P, 2, P), np.float32)
    ipair[:, 0, :] = np.eye(P)
    sel2 = np.zeros((2, P), np.float32)
    sel2[0, 0:64] = 1.0
    sel2[1, 64:128] = 1.0

    common = {
        "wq": tile_in_out(WQ8 * Wq_f, 8, 4, osz=256),
        "wk": tile_in_out(WK8 * Wk_f, 8, 4, osz=256),
        "wv": tile_in_out(WV8 * Wv_f, 8, 4, osz=256),
        "wp": np.ascontiguousarray(
            (WP8 * Wproj).reshape(8, P, C).transpose(1, 0, 2).astype(f8l)),
        "w1": tile_in_out(W18 * W1_f, 8, 32),
        "w2": tile_in_out(W28 * W2, 32, 8),
        "ball": np.ascontiguousarray(np.concatenate([
            bq_f.reshape(8, P).T, bk_f.reshape(8, P).T,
            b1_f.reshape(32, P).T], axis=1)),
        "bprow": np.ascontiguousarray(
            (WP8 * bp_f).reshape(1, 8, P).astype(bfl)),
        "b2row": np.ascontiguousarray(
            (W28 * np.asarray(b2, np.float32)).reshape(1, 8, P).astype(bfl)),
        "ipair": np.ascontiguousarray(ipair.astype(f8l)),
        "sel2": np.ascontiguousarray(sel2),
        "ones128": np.ones((1, P), np.float32),
        "eights128": np.full((1, P), 8.0, np.float32),
        "onesrow": np.ones((1, 512), bfl),
        "onescolbf": np.ones((P, 1), bfl),
    }

    in_maps = []
    row_maps = []
    for core in range(8):
        b, s = core // 2, core % 2
        rows = np.concatenate(
            [np.arange(512 * i + 256 * s, 512 * i + 256 * s + 256)
             for i in range(4)])
        row_maps.append((b, rows))
        xb = x[b]
        if s == 1:
            # swap the 256-halves of every 512-token chunk so this core's
            # own (odd-block) queries sit at chunk columns [0:256)
            perm = np.concatenate(
                [np.concatenate([np.arange(512 * i + 256, 512 * i + 512),
                                 np.arange(512 * i, 512 * i + 256)])
                 for i in range(4)])
            xkv = xb[perm]
        else:
            xkv = xb
        xq = np.ascontiguousarray(xb[rows])
        in_maps.append({
            "xkvt": np.ascontiguousarray(xkv.T.reshape(8, P, T).astype(bfl)),
            "xqres": np.ascontiguousarray(xq.T.reshape(8, P, TQ).astype(bfl)),
            "sbias": sbias_sets[s],
            **common,
        })
    return in_maps, row_maps


def kernel(**inputs):
    nc = _get_program()
    in_maps, row_maps = _host_prep(**inputs)
    res = run_bass_kernel_spmd(nc, in_maps, core_ids=list(range(8)))
    out = np.empty((B, T, C), np.float32)
    for core in range(8):
        b, rows = row_maps[core]
        out[b][rows] = res.results[core]["y"].T
    return out


if __name__ == "__main__":
    from concourse.timeline_sim import TimelineSim
    print("sim ns:", TimelineSim(build_program(), trace=False).simulate())


# revision 95
# speedup vs baseline: 1.0068x; 1.0050x over previous
"""Trainium2 Bass kernel for one dense transformer block.

Full (unsharded) IO: x [4, 2048, 1024] -> out [4, 2048, 1024].
Sharding: 8 cores = 4 batches x 2 query sets. Each core owns one batch's K/V
(2048 rows) and 1024 query rows. Set 0 takes even 256-row blocks {0,2,4,6},
set 1 odd blocks {1,3,5,7}. Set-1 cores store their keys with the two
256-halves of every 512-token chunk swapped, so every core's own query rows
sit at columns [0:256) of each 512-chunk of its key layout -- the Q
projection reads the LN output directly (no separate q-side LayerNorm), and
the causal boundary tiles sit at the same slot positions on every core
(masks are per-core data; the instruction stream is identical).

Numerics: fp8e4 DoubleRow matmuls (0.5 cyc/col, 256-deep contraction) for
Q/K/V/AV/proj/FFN; S stays bf16. Causal masking is additive: a -30 bias is
preloaded into the S psum via one fp8-DR identity matmul, so exp() output
is written as fp8 directly and feeds DoubleRow AV matmuls. The softmax
denominator rides as a ones-column in the V tiles (padded to 128-wide DR
weights; the pad rows' outputs are never read); normalization is a DVE
reciprocal + PE broadcast + DVE multiply per (head, query-block). proj/FFN2
biases are added via rank-1 bf16 matmuls into the psum. The post-attention
tail (proj -> LN2 -> FFN) is column-pipelined so LN2 chains overlap proj/FFN
matmuls.
"""

import sys

sys.path.insert(0, "/opt/trn_rl_repo")

import numpy as np

import concourse.bass as bass
import concourse.mybir as mybir
import concourse.tile as tile
from concourse.bass_utils import run_bass_kernel_spmd

f32 = mybir.dt.float32
f32r = mybir.dt.float32r
bf16 = mybir.dt.bfloat16
fp8 = mybir.dt.float8e4
AL = mybir.AluOpType
AF = mybir.ActivationFunctionType
DR = mybir.MatmulPerfMode.DoubleRow

B, T, C = 4, 2048, 1024
H, D = 16, 64
F = 4 * C
P = 128
TQ = 1024            # query rows per core
LN_EPS = 1e-5
H8 = 8.0             # LN-output fp8 scale
WK8, WQ8, WV8 = 64.0, 512.0, 64.0
WP8, W18, W28 = 64.0, 16.0, 32.0


def _split_sync_waits(nc):
    """This container's walrus supports one sync-wait per instruction; Tile
    emits up to ~3. Hoist extras onto NoOps inserted before the owner."""
    ctr = 0
    for fn in nc.m.functions:
        for bb in fn.blocks:
            out, changed = [], False
            for ins in bb.instructions:
                si = ins.sync_info
                waits = list(si.on_wait) if si is not None and si.on_wait else []
                if len(waits) > 1:
                    changed = True
                    for w in waits[:-1]:
                        ctr += 1
                        nop = mybir.InstNoOp(name=f"waitsplit_{ctr}", ins=[], outs=[])
                        nop.engine = ins.engine
                        nop.sync_info = mybir.SyncInfo(on_wait=[w], on_update=[])
                        out.append(nop)
                        nc.register_instruction(nop, overwrite=True)
                    ins.sync_info = mybir.SyncInfo(
                        on_wait=[waits[-1]], on_update=list(si.on_update or [])
                    )
                out.append(ins)
            if changed:
                bb.instructions = out


def build_program():
    nc = bass.Bass()
    xkvt_d = nc.dram_tensor("xkvt", [8, P, T], bf16, kind="ExternalInput")
    xqres_d = nc.dram_tensor("xqres", [8, P, TQ], bf16, kind="ExternalInput")
    sbias_d = nc.dram_tensor("sbias", [P, 2, 1024], fp8, kind="ExternalInput")
    ipair_d = nc.dram_tensor("ipair", [P, 2, P], fp8, kind="ExternalInput")
    ones128_d = nc.dram_tensor("ones128", [1, P], f32r, kind="ExternalInput")
    eights128_d = nc.dram_tensor("eights128", [1, P], f32r, kind="ExternalInput")
    onescol_bf_d = nc.dram_tensor("onescolbf", [P, 1], bf16, kind="ExternalInput")
    sel2_d = nc.dram_tensor("sel2", [2, P], f32r, kind="ExternalInput")
    onesrow_d = nc.dram_tensor("onesrow", [1, 512], bf16, kind="ExternalInput")
    bprow_d = nc.dram_tensor("bprow", [1, 8, P], bf16, kind="ExternalInput")
    b2row_d = nc.dram_tensor("b2row", [1, 8, P], bf16, kind="ExternalInput")
    wq_d = nc.dram_tensor("wq", [4, P, 8, 256], fp8, kind="ExternalInput")
    wk_d = nc.dram_tensor("wk", [4, P, 8, 256], fp8, kind="ExternalInput")
    wv_d = nc.dram_tensor("wv", [4, P, 8, 256], fp8, kind="ExternalInput")
    wp_d = nc.dram_tensor("wp", [P, 8, C], fp8, kind="ExternalInput")
    w1_d = nc.dram_tensor("w1", [32, P, 8, P], fp8, kind="ExternalInput")
    w2_d = nc.dram_tensor("w2", [8, P, 32, P], fp8, kind="ExternalInput")
    ball_d = nc.dram_tensor("ball", [P, 48], f32, kind="ExternalInput")
    y_d = nc.dram_tensor("y", [C, TQ], f32, kind="ExternalOutput")

    with tile.TileContext(nc) as tc:
        with tc.tile_pool(name="consts", bufs=1) as cpool, \
             tc.tile_pool(name="pers", bufs=1) as pers, \
             tc.tile_pool(name="proj_in", bufs=1) as prpool:
            ones128 = cpool.tile([1, P], f32r)
            nc.sync.dma_start(ones128, ones128_d[:, :])
            eights128 = cpool.tile([1, P], f32r)
            nc.sync.dma_start(eights128, eights128_d[:, :])
            onescol_bf = cpool.tile([P, 1], bf16)
            nc.sync.dma_start(onescol_bf, onescol_bf_d[:, :])
            sel2 = cpool.tile([2, P], f32r)
            nc.sync.dma_start(sel2, sel2_d[:, :])
            onesrow = cpool.tile([1, 512], bf16)
            nc.sync.dma_start(onesrow, onesrow_d[:, :])
            bprow = cpool.tile([1, 8, P], bf16)
            nc.sync.dma_start(bprow, bprow_d[:, :, :])
            b2row = cpool.tile([1, 8, P], bf16)
            nc.sync.dma_start(b2row, b2row_d[:, :, :])
            ipair = cpool.tile([P, 2, P], fp8)
            nc.sync.dma_start(ipair, ipair_d[:, :, :])
            sbias = cpool.tile([P, 2, 1024], fp8)
            eps1 = cpool.tile([1, 1], f32)
            nc.vector.memset(eps1, LN_EPS)
            ball = cpool.tile([P, 48], f32)
            bq_sb = ball[:, 0:8]
            bk_sb = ball[:, 8:16]
            b1_sb = ball[:, 16:48]

            OT8 = pers.tile([P, 8, TQ], fp8)       # attn out / den, fp8

            def ln_stats(src, cols, lnps, lnsm, lnsb, ptag=None, big_pool=None,
                         sq_dve=False, sqp=None):
                """Sum / sum-of-squares ones-matmuls over one 512-col chunk
                of a [P, 8, N] bf16 tensor, then the small chain down to the
                rstd / mean*rstd row vectors."""
                if big_pool is not None:
                    psum_smt = big_pool.tile([P, 4, 256], f32, tag="s",
                                             name="psum_smt")
                    psum_sqt = big_pool.tile([P, 4, 256], f32, tag="s",
                                             name="psum_sqt")
                    psum_sm = psum_smt[0:1, 0:2, :]
                    psum_sq = psum_sqt[0:1, 0:2, :]
                else:
                    psum_sm = lnps.tile([1, 512], f32, tag=ptag or "sm",
                                        name="psum_sm")
                    psum_sq = lnps.tile([1, 512], f32, tag=ptag or "sq",
                                        name="psum_sq")
                for ft in range(8):
                    sq = (sqp or lnsb).tile([P, 512], bf16, tag="sq")
                    if sq_dve:
                        nc.vector.tensor_tensor(
                            sq, src[:, ft, cols], src[:, ft, cols], AL.mult)
                    else:
                        nc.scalar.activation(sq, src[:, ft, cols], AF.Square)
                    nc.tensor.matmul(psum_sm, onescol_bf, src[:, ft, cols],
                                     start=(ft == 0), stop=(ft == 7))
                    nc.tensor.matmul(psum_sq, onescol_bf, sq,
                                     start=(ft == 0), stop=(ft == 7))
                mean = lnsm.tile([1, 512], f32, tag="mean")
                nc.vector.tensor_scalar_mul(mean, psum_sm, 1.0 / C)
                msq = lnsm.tile([1, 512], f32, tag="msq")
                nc.vector.tensor_scalar_mul(msq, psum_sq, 1.0 / C)
                var = lnsm.tile([1, 512], f32, tag="var")
                nc.vector.tensor_tensor(var, mean, mean, AL.mult)
                nc.vector.tensor_tensor(var, msq, var, AL.subtract)
                rrow = lnsm.tile([1, 512], f32r, tag="rrow")
                nc.scalar.activation(rrow, var, AF.Sqrt, bias=eps1, scale=1.0)
                with nc.allow_low_precision(reason="f32r has f32 bits"):
                    nc.vector.reciprocal(rrow, rrow)
                mrow = lnsm.tile([1, 512], f32r, tag="mrow")
                nc.vector.tensor_tensor(mrow, mean, rrow, AL.mult)
                return rrow, mrow

            def ln_finish(src, dst, cols, rows, lnps, lnsb, ptag=None,
                          pool_split=True):
                """Broadcast the (x8) row vectors along partitions and apply;
                dst is fp8 (x8-scaled LN output)."""
                rrow, mrow = rows
                psum_r = lnps.tile([P, 512], f32, tag=ptag or "bc",
                                   name="psum_r")
                nc.tensor.matmul(psum_r, eights128, rrow, start=True, stop=True)
                psum_m = lnps.tile([P, 512], f32, tag=ptag or "bc",
                                   name="psum_m")
                nc.tensor.matmul(psum_m, eights128, mrow, start=True, stop=True)
                rbc = lnsb.tile([P, 512], bf16, tag="rbc")
                nc.scalar.copy(rbc, psum_r)
                mbc = lnsb.tile([P, 512], bf16, tag="mbc")
                nc.scalar.copy(mbc, psum_m)
                for ft in range(8):
                    tmp = lnsb.tile([P, 512], bf16, tag="tmp")
                    k = 2 if pool_split else 4
                    eng = nc.gpsimd if ft % k == 0 else nc.vector
                    eng.tensor_tensor(
                        tmp, src[:, ft, cols], rbc, AL.mult)
                    eng2 = nc.gpsimd if ft % k == k - 1 else nc.vector
                    eng2.tensor_tensor(
                        dst[:, ft, cols], tmp, mbc, AL.subtract)

            # ------------- Phase 0: load xT; LN1 -> h8 (fp8, x8) -------------
            with tc.tile_pool(name="xt_sb", bufs=1) as xtpool:
                wp_t = prpool.tile([P, 8, C], fp8)
                xqT = prpool.tile([P, 8, TQ], bf16)
                xTc = [xtpool.tile([P, 8, 512], bf16, name=f"xtc{cc}")
                       for cc in range(4)]
                h8c = [prpool.tile([P, 8, 512], fp8, name=f"h8c{cc}")
                       for cc in range(4)]

                # ---------------- Phase A: attention ----------------
                with tc.tile_pool(name="wk_p", bufs=2) as wkp, \
                     tc.tile_pool(name="wq_p", bufs=2) as wqp, \
                     tc.tile_pool(name="wv_p", bufs=1) as wvp, \
                     tc.tile_pool(name="kq_big", bufs=2) as gpool, \
                     tc.tile_pool(name="v_sb", bufs=2) as vpool, \
                     tc.tile_pool(name="pt_sb", bufs=12) as ptpool, \
                     tc.tile_pool(name="den_sb", bufs=1) as dpool, \
                     tc.tile_pool(name="ps_ab", bufs=2, space="PSUM") as ps_ab, \
                     tc.tile_pool(name="ps_s", bufs=2, space="PSUM") as ps_s, \
                     tc.tile_pool(name="ps_o", bufs=2, space="PSUM") as ps_o:

                    def make_units(gp):
                        """Prefetch gp's weights now; return (tiles, unit
                        closures) emitting gp's K/Q/V DoubleRow pipelines."""
                        wk_t = wkp.tile([P, 8, 256], fp8, tag="wk")
                        nc.sync.dma_start(wk_t, wk_d[gp])
                        wq_t = wqp.tile([P, 8, 256], fp8, tag="wq")
                        nc.sync.dma_start(wq_t, wq_d[gp])
                        wv_t = wvp.tile([P, 8, 256], fp8, tag="wv")
                        nc.sync.dma_start(wv_t, wv_d[gp])
                        KT2 = gpool.tile([P, 2, T], bf16, tag="KT2")
                        QT2 = gpool.tile([P, 2, TQ], bf16, tag="QT2")
                        vaug = vpool.tile([P, 16, 4, P], fp8, tag="vaug")
                        units = []

                        def u_vones():
                            # col 64 is the softmax-denominator ones column;
                            # cols 65:128 pad DR weights to M=128 (their out
                            # rows are never read)
                            nc.gpsimd.memset(vaug[:, :, :, 64:P], 1.0)
                        units.append(u_vones)

                        def mk_k(gi, rc):
                            def u():
                                g = 2 * gp + gi
                                cols = slice(rc * 512, (rc + 1) * 512)
                                psum_k = ps_ab.tile([P, 512], f32, tag="ab")
                                for k in range(4):
                                    nc.tensor.matmul(
                                        psum_k,
                                        wk_t[:, 2 * k:2 * k + 2,
                                             128 * gi:128 * (gi + 1)],
                                        h8c[rc][:, 2 * k:2 * k + 2, :],
                                        start=(k == 0), stop=(k == 3),
                                        perf_mode=DR)
                                nc.vector.tensor_scalar(
                                    KT2[:, gi, cols], psum_k,
                                    1.0 / (WK8 * H8), bk_sb[:, g:g + 1],
                                    op0=AL.mult, op1=AL.add)
                            return u

                        def mk_q(gi, cc):
                            def u():
                                g = 2 * gp + gi
                                cols = slice(cc * 256, (cc + 1) * 256)
                                psum_q = ps_ab.tile([P, 256], f32, tag="ab")
                                for k in range(4):
                                    nc.tensor.matmul(
                                        psum_q,
                                        wq_t[:, 2 * k:2 * k + 2,
                                             128 * gi:128 * (gi + 1)],
                                        h8c[cc][:, 2 * k:2 * k + 2, 0:256],
                                        start=(k == 0), stop=(k == 3),
                                        perf_mode=DR)
                                nc.vector.tensor_scalar(
                                    QT2[:, gi, cols], psum_q,
                                    1.0 / (WQ8 * H8), bq_sb[:, g:g + 1],
                                    op0=AL.mult, op1=AL.add)
                            return u

                        def mk_v(kt):
                            def u():
                                psum_v = ps_ab.tile([P, 4, 64], f32, tag="ab")
                                cc, t = kt // 4, kt % 4
                                for k in range(4):
                                    nc.tensor.matmul(
                                        psum_v,
                                        h8c[cc][:, 2 * k:2 * k + 2,
                                                t * P:(t + 1) * P],
                                        wv_t[:, 2 * k:2 * k + 2, :],
                                        start=(k == 0), stop=(k == 3),
                                        perf_mode=DR)
                                nc.vector.tensor_scalar_mul(
                                    vaug[:, kt, :, 0:64], psum_v,
                                    1.0 / (WV8 * H8))
                            return u

                        for rc in range(4):
                            units.append(mk_k(0, rc))
                            units.append(mk_k(1, rc))
                        for cc in range(4):
                            units.append(mk_q(0, cc))
                            units.append(mk_q(1, cc))
                        for kt in range(16):
                            units.append(mk_v(kt))
                        return (KT2, QT2, vaug), units

                    def emit_gp(gp, tiles, units_next, ustart=0, lag=10):
                        """Emit gp's S/exp/AV stream with a one-batch AV lag,
                        interleaving the next gp's K/Q/V units into the exp
                        latency gaps."""
                        KT2, QT2, vaug = tiles
                        batches = []
                        for pos in range(4):
                            for gi in range(2):
                                for hh in range(2):
                                    for done in range(0, 4 * pos + 4, 4):
                                        batches.append((gi, pos, hh, done))
                        pending = []
                        psum_os = {}
                        den_ps = {}
                        ui = 0

                        def emit_av(batch, pt):
                            gi, pos, hh, done = batch
                            nkt = 4 * pos + 4
                            qc = slice(pos * 256, (pos + 1) * 256)
                            if (gi, pos) not in psum_os:
                                psum_os[(gi, pos)] = ps_o.tile(
                                    [P, 2, 256], f32, tag="o",
                                    name=f"psum_o_{gp}_{gi}_{pos}")
                            if gi not in den_ps:
                                den_ps[gi] = tuple(
                                    dpool.tile([1, TQ], f32r,
                                               tag=f"den{gi}{h2}",
                                               name=f"den_{gp}_{gi}_{h2}")
                                    for h2 in range(2))
                            psum_o = psum_os[(gi, pos)]
                            for t in range(2):
                                kt0 = done + 2 * t
                                nc.tensor.matmul(
                                    psum_o[:, hh, :],
                                    vaug[:, kt0:kt0 + 2, 2 * gi + hh, 0:P],
                                    pt[:, 2 * t:2 * t + 2, :],
                                    start=(kt0 == 0), stop=(kt0 + 2 == nkt),
                                    perf_mode=DR)
                            if done + 4 == nkt and hh == 1:
                                den_p = den_ps[gi]
                                g = 2 * gp + gi
                                bc_sb = dpool.tile(
                                    [P, 2, 256], bf16, tag="bcs",
                                    name=f"bcs_{gp}_{gi}_{pos}")
                                for h2 in range(2):
                                    with nc.allow_low_precision(
                                            reason="f32r has f32 bits"):
                                        nc.vector.reciprocal(
                                            den_p[h2][:, qc],
                                            psum_o[64:65, h2, :])
                                    psum_bc = ps_ab.tile(
                                        [64, 256], f32, tag="ab",
                                        name=f"bc_{gp}_{gi}_{pos}_{h2}")
                                    nc.tensor.matmul(
                                        psum_bc, ones128[:, 0:64],
                                        den_p[h2][:, qc],
                                        start=True, stop=True)
                                    nc.vector.tensor_copy(
                                        bc_sb[0:64, h2, :], psum_bc)
                                for h2 in range(2):
                                    nc.vector.tensor_tensor(
                                        OT8[64 * h2:64 * h2 + 64, g, qc],
                                        psum_o[0:64, h2, :],
                                        bc_sb[0:64, h2, :],
                                        AL.mult)

                        for i, batch in enumerate(batches):
                            gi, pos, hh, done = batch
                            nkt = 4 * pos + 4
                            final = (done + 4 == nkt)
                            qc = slice(pos * 256, (pos + 1) * 256)
                            hb = slice(64 * hh, 64 * hh + 64)
                            psum_s = ps_s.tile([P, 4, 256], f32, tag="s")
                            if final:
                                for bh in range(2):
                                    nc.tensor.matmul(
                                        psum_s[:, 2 * bh:2 * bh + 2, :], ipair,
                                        sbias[:, :, 512 * bh:512 * (bh + 1)],
                                        start=True, stop=False,
                                        perf_mode=DR, skip_group_check=True)
                            for j in range(4):
                                kt = done + j
                                nc.tensor.matmul(
                                    psum_s[:, j, :],
                                    KT2[hb, gi, kt * P:(kt + 1) * P],
                                    QT2[hb, gi, qc],
                                    start=(not final), stop=True,
                                    skip_group_check=True)
                            pt = ptpool.tile([P, 4, 256], fp8, tag="pt")
                            nc.scalar.activation(pt, psum_s, AF.Exp)
                            pending.append((batch, pt))
                            if len(pending) > lag:
                                emit_av(*pending.pop(0))
                            if i + 1 > ustart:
                                target = ((i + 1 - ustart) * len(units_next)
                                          // (len(batches) - ustart))
                            else:
                                target = 0
                            while ui < target:
                                units_next[ui]()
                                ui += 1
                        while pending:
                            emit_av(*pending.pop(0))
                        while ui < len(units_next):
                            units_next[ui]()
                            ui += 1

                    with tc.tile_pool(name="sq_sb", bufs=6) as sqpool, \
                         tc.tile_pool(name="ln_sb", bufs=4) as lnsb, \
                         tc.tile_pool(name="ln_small", bufs=2) as lnsm:
                        for ft in range(8):
                            nc.sync.dma_start(xTc[0][:, ft, :],
                                              xkvt_d[ft][:, 0:512])
                        nc.sync.dma_start(ball, ball_d[:, :])
                        nc.sync.dma_start(sbias, sbias_d[:, :, :])
                        for cc in range(1, 4):
                            for ft in range(8):
                                nc.sync.dma_start(
                                    xTc[cc][:, ft, :],
                                    xkvt_d[ft][:, cc * 512:(cc + 1) * 512])
                        tiles_cur, units_cur = make_units(0)
                        # unit index map: 0=vones, 1+2rc+gi=K, 9+2cc+gi=Q,
                        # 17+kt=V; K/Q/V units for chunk cc depend on h8c[cc]
                        units_cur[0]()
                        cunits = [(cc, [units_cur[1 + 2 * cc],
                                        units_cur[2 + 2 * cc],
                                        units_cur[9 + 2 * cc],
                                        units_cur[10 + 2 * cc]] +
                                   [units_cur[17 + kt]
                                    for kt in range(4 * cc, 4 * cc + 4)])
                                  for cc in range(4)]
                        half = slice(0, 512)
                        # two-deep software pipeline: stats(cc+1) overlaps
                        # finish(cc)'s chain; units run after their chunk
                        rows_q = [ln_stats(xTc[0], half, ps_ab, lnsm, lnsb,
                                           ptag="ab", big_pool=ps_s,
                                           sqp=sqpool)]
                        for cc in range(4):
                            if cc + 1 < 4:
                                rows_q.append(
                                    ln_stats(xTc[cc + 1], half, ps_ab, lnsm,
                                             lnsb, ptag="ab", big_pool=ps_s,
                                             sqp=sqpool))
                            ln_finish(xTc[cc], h8c[cc], half, rows_q[cc],
                                      ps_ab, lnsb, ptag="ab",
                                      pool_split=(cc < 2))
                            if cc < 2:
                                for u in cunits[cc][1]:
                                    u()
                        carry = []
                        for cc in (2, 3):
                            us = cunits[cc][1]
                            carry.extend([us[2], us[3], us[0], us[1]] + us[4:])
                        for gp in range(4):
                            if gp == 1:
                                # proj inputs: DMA is idle once phase 0's
                                # xkvt/weight burst drains
                                nc.sync.dma_start(wp_t, wp_d[:, :, :])
                                for ft in range(8):
                                    nc.sync.dma_start(xqT[:, ft, :],
                                                      xqres_d[ft])
                            if gp < 3:
                                tiles_next, units_next = make_units(gp + 1)
                            else:
                                tiles_next, units_next = None, []
                            if gp == 0:
                                units_next = carry + units_next
                            emit_gp(gp, tiles_cur, units_next,
                                    ustart=(10 if gp == 0 else 8),
                                    lag=10)
                            tiles_cur = tiles_next

            # ---------------- Phase B: proj + residual + LN2 ----------------
            with tc.tile_pool(name="late_pers", bufs=1) as late:
                x2T = late.tile([P, 8, TQ], bf16)      # post-proj residual
                h2f8 = late.tile([P, 8, TQ], fp8)      # LN2 output (x8, fp8)
                with tc.tile_pool(name="ln2_sb", bufs=5) as lnsb2, \
                     tc.tile_pool(name="ln2_small", bufs=2) as lnsm2, \
                     tc.tile_pool(name="ln2_ps", bufs=1, space="PSUM") as lnps2, \
                     tc.tile_pool(name="w1_sb", bufs=8) as w1pool, \
                     tc.tile_pool(name="w2_sb", bufs=8) as w2pool, \
                     tc.tile_pool(name="relu_sb", bufs=1) as rpool, \
                     tc.tile_pool(name="y_sb", bufs=4) as ypool, \
                     tc.tile_pool(name="ps_f1", bufs=3, space="PSUM") as ps_f1, \
                     tc.tile_pool(name="ps_f2", bufs=2, space="PSUM") as ps_f2:
                    relu1T = rpool.tile([P, 32, TQ], fp8)

                    def proj_unit(of, rc):
                        cols = slice(rc * 512, (rc + 1) * 512)
                        psum_p = ps_f1.tile([P, 512], f32, tag="f1", name=f"pp_{of}_{rc}")
                        nc.tensor.matmul(
                            psum_p, bprow[:, of, :], onesrow,
                            start=True, stop=False,
                            skip_group_check=True)
                        for k in range(4):
                            nc.tensor.matmul(
                                psum_p,
                                wp_t[:, 2 * k:2 * k + 2,
                                     of * P:(of + 1) * P],
                                OT8[:, 2 * k:2 * k + 2, cols],
                                start=False, stop=(k == 3),
                                perf_mode=DR, skip_group_check=True)
                        nc.vector.scalar_tensor_tensor(
                            x2T[:, of, cols], psum_p, 1.0 / WP8,
                            xqT[:, of, cols], op0=AL.mult, op1=AL.add)

                    w1_ts, w2_ts = {}, {}

                    def w1_load(fk, rc):
                        w1_ts[(fk, rc)] = w1pool.tile(
                            [P, 8, P], fp8, tag="w1", name=f"w1_{rc}_{fk}")
                        nc.sync.dma_start(w1_ts[(fk, rc)], w1_d[fk])

                    def w2_load(of, rc):
                        if (of, 0) in w2_ts:
                            w2_ts[(of, rc)] = w2_ts[(of, 0)]
                            return
                        w2_ts[(of, rc)] = w2pool.tile(
                            [P, 32, P], fp8, tag="w2", name=f"w2_{rc}_{of}")
                        nc.sync.dma_start(w2_ts[(of, rc)], w2_d[of])

                    def ffn1_unit(fk, rc):
                        w1_t = w1_ts.pop((fk, rc))
                        cols = slice(rc * 512, (rc + 1) * 512)
                        psum_f = ps_f1.tile([P, 512], f32, tag="f1")
                        for kk in range(4):
                            nc.tensor.matmul(
                                psum_f,
                                w1_t[:, 2 * kk:2 * kk + 2, :],
                                h2f8[:, 2 * kk:2 * kk + 2, cols],
                                start=(kk == 0), stop=(kk == 3),
                                perf_mode=DR)
                        nc.scalar.activation(
                            relu1T[:, fk, cols], psum_f, AF.Relu,
                            bias=b1_sb[:, fk:fk + 1],
                            scale=1.0 / (W18 * H8))

                    def ffn2_unit(of, rc):
                        w2_t = w2_ts[(of, rc)]
                        cols = slice(rc * 512, (rc + 1) * 512)
                        psum_f2 = ps_f2.tile([P, 512], f32, tag="f2")
                        nc.tensor.matmul(
                            psum_f2, b2row[:, of, :], onesrow,
                            start=True, stop=False,
                            skip_group_check=True)
                        for kk in range(16):
                            nc.tensor.matmul(
                                psum_f2,
                                w2_t[:, 2 * kk:2 * kk + 2, :],
                                relu1T[:, 2 * kk:2 * kk + 2, cols],
                                start=False, stop=(kk == 15),
                                perf_mode=DR, skip_group_check=True)
                        y_sb = ypool.tile([P, 512], f32, tag="y")
                        nc.vector.scalar_tensor_tensor(
                            y_sb, psum_f2, 1.0 / W28,
                            x2T[:, of, cols], op0=AL.mult, op1=AL.add)
                        nc.scalar.dma_start(
                            y_d[of * P:(of + 1) * P, cols], y_sb)

                    # column-pipelined tail: proj rc1 overlaps LN2(c0)'s
                    # chain; FFN1 rc0 overlaps LN2(c1); FFN2 rc0 overlaps
                    # FFN1 rc1. Weight DMAs prefetched two units ahead.
                    for of in range(8):
                        w2_load(of, 0)
                    for of in range(8):
                        proj_unit(of, 0)
                    c0 = slice(0, 512)
                    rows0 = ln_stats(x2T, c0, lnps2, lnsm2, lnsb2)
                    for of in range(8):
                        proj_unit(of, 1)
                    ln_finish(x2T, h2f8, c0, rows0, lnps2, lnsb2)
                    c1 = slice(512, 1024)
                    rows1 = ln_stats(x2T, c1, lnps2, lnsm2, lnsb2)

                    sched = [("f1", fk, 0) for fk in range(32)]
                    for fk in range(32):
                        sched.append(("f1", fk, 1))
                        if fk % 4 == 0:
                            sched.append(("f2", fk // 4, 0))
                    sched += [("f2", of, 1) for of in range(8)]
                    loads = {"f1": w1_load, "f2": w2_load}
                    units = {"f1": ffn1_unit, "f2": ffn2_unit}
                    for i in range(6):
                        loads[sched[i][0]](*sched[i][1:])
                    ln1_done = False
                    for i, (kind, a, b) in enumerate(sched):
                        if i + 6 < len(sched):
                            nxt = sched[i + 6]
                            loads[nxt[0]](*nxt[1:])
                        units[kind](a, b)
                        if i == 7 and not ln1_done:
                            ln_finish(x2T, h2f8, c1, rows1, lnps2, lnsb2)
                            ln1_done = True
    _split_sync_waits(nc)
    return nc


_PROGRAM = None


def _get_program():
    global _PROGRAM
    if _PROGRAM is None:
        _PROGRAM = build_program()
    return _PROGRAM


def _host_prep(x, Wk, Wq, Wv, Wproj, bproj, W1, b1, W2, b2, g1, beta1, g2, beta2):
    """Fold LN affines into weights; build per-core shards (host work is
    layout marshalling only -- all input-dependent math runs on device)."""
    import ml_dtypes

    bfl = ml_dtypes.bfloat16
    f8l = ml_dtypes.float8_e4m3
    x = np.asarray(x, np.float32)
    Wq = np.asarray(Wq, np.float32)
    Wk = np.asarray(Wk, np.float32)
    Wv = np.asarray(Wv, np.float32)
    Wproj = np.asarray(Wproj, np.float32)
    W1 = np.asarray(W1, np.float32)
    W2 = np.asarray(W2, np.float32)
    g1 = np.asarray(g1, np.float32)
    beta1 = np.asarray(beta1, np.float32)
    g2 = np.asarray(g2, np.float32)
    beta2 = np.asarray(beta2, np.float32)

    scale = 1.0 / np.sqrt(D)
    Wq_f = (g1[:, None] * Wq) * scale
    bq_f = (beta1 @ Wq) * scale
    Wk_f = g1[:, None] * Wk
    bk_f = beta1 @ Wk
    Wv_f = g1[:, None] * Wv
    bv_f = beta1 @ Wv
    # softmax rows sum to 1 -> the V bias lands as a constant on O; fold it
    # into the proj bias
    bp_f = np.asarray(bproj, np.float32) + bv_f @ Wproj
    W1_f = g2[:, None] * W1
    b1_f = np.asarray(b1, np.float32) + beta2 @ W1

    def tile_in_out(W, n_in, n_out, osz=P, dt=f8l):
        # [in, out] -> [n_out, 128, n_in, osz]
        return np.ascontiguousarray(
            W.reshape(n_in, P, n_out, osz).transpose(2, 1, 0, 3).astype(dt))

    # additive causal bias tiles for the final 4-kt group: slot 0 = tri0,
    # slot 1 = tri1; slots 2,3: -30 for set 0 (beyond its queries), 0 for
    # set 1 (fully-allowed earlier keys in its swapped layout)
    tri0 = (np.arange(256)[None, :] >= np.arange(P)[:, None])
    tri1 = (np.arange(256)[None, :] >= np.arange(P)[:, None] + 128)
    sbias_sets = []
    for s in range(2):
        m = np.zeros((P, 2, 4, 256), np.float32)
        m[:, 0, 0, :] = np.where(tri0, 0.0, -30.0)
        m[:, 0, 1, :] = np.where(tri1, 0.0, -30.0)
        if s == 0:
            m[:, 0, 2, :] = -30.0
            m[:, 0, 3, :] = -30.0
        sbias_sets.append(m.reshape(P, 2, 1024).astype(f8l))

    ipair = np.zeros((P, 2, P), np.float32)
    ipair[:, 0, :] = np.eye(P)
    sel2 = np.zeros((2, P), np.float32)
    sel2[0, 0:64] = 1.0
    sel2[1, 64:128] = 1.0

    common = {
        "wq": tile_in_out(WQ8 * Wq_f, 8, 4, osz=256),
        "wk": tile_in_out(WK8 * Wk_f, 8, 4, osz=256),
        "wv": tile_in_out(WV8 * Wv_f, 8, 4, osz=256),
        "wp": np.ascontiguousarray(
            (WP8 * Wproj).reshape(8, P, C).transpose(1, 0, 2).astype(f8l)),
        "w1": tile_in_out(W18 * W1_f, 8, 32),
        "w2": tile_in_out(W28 * W2, 32, 8),
        "ball": np.ascontiguousarray(np.concatenate([
            bq_f.reshape(8, P).T, bk_f.reshape(8, P).T,
            b1_f.reshape(32, P).T], axis=1)),
        "bprow": np.ascontiguousarray(
            (WP8 * bp_f).reshape(1, 8, P).astype(bfl)),
        "b2row": np.ascontiguousarray(
            (W28 * np.asarray(b2, np.float32)).reshape(1, 8, P).astype(bfl)),
        "ipair": np.ascontiguousarray(ipair.astype(f8l)),
        "sel2": np.ascontiguousarray(sel2),
        "ones128": np.ones((1, P), np.float32),
        "eights128": np.full((1, P), 8.0, np.float32),
        "onesrow": np.ones((1, 512), bfl),
        "onescolbf": np.ones((P, 1), bfl),
    }

    in_maps = []
    row_maps = []
    for core in range(8):
        b, s = core // 2, core % 2
        rows = np.concatenate(
            [np.arange(512 * i + 256 * s, 512 * i + 256 * s + 256)
             for i in range(4)])
        row_maps.append((b, rows))
        xb = x[b]
        if s == 1:
            # swap the 256-halves of every 512-token chunk so this core's
            # own (odd-block) queries sit at chunk columns [0:256)
            perm = np.concatenate(
                [np.concatenate([np.arange(512 * i + 256, 512 * i + 512),
                                 np.arange(512 * i, 512 * i + 256)])
                 for i in range(4)])
            xkv = xb[perm]
        else:
            xkv = xb
        xq = np.ascontiguousarray(xb[rows])
        in_maps.append({
            "xkvt": np.ascontiguousarray(xkv.T.reshape(8, P, T).astype(bfl)),
            "xqres": np.ascontiguousarray(xq.T.reshape(8, P, TQ).astype(bfl)),
            "sbias": sbias_sets[s],
            **common,
        })
    return in_maps, row_maps


def kernel(**inputs):
    nc = _get_program()
    in_maps, row_maps = _host_prep(**inputs)
    res = run_bass_kernel_spmd(nc, in_maps, core_ids=list(range(8)))
    out = np.empty((B, T, C), np.float32)
    for core in range(8):
        b, rows = row_maps[core]
        out[b][rows] = res.results[core]["y"].T
    return out


if __name__ == "__main__":
    from concourse.timeline_sim import TimelineSim
    print("sim ns:", TimelineSim(build_program(), trace=False).simulate())
